# revision 14
# baseline (speedup 1.0000x reference)
"""Trainium2 Bass kernel for nn_ActorCritic (3-layer edge-GNN, qconv stack).

Strategy (8 NeuronCores):
  - Nodes sharded 8 ways by dst: core c owns nodes [c*6250, (c+1)*6250).
  - Edges assigned to the core owning their dst; segment-sum is core-local.
  - Per layer l: u_l = h_{l-1} @ W1_l[:, :F].T is computed per-core on own
    nodes and AllGathered into a DRAM table; per-edge work is
      t_e = leaky_relu(u_l[src_e] + v_e),   v_e = w_e @ W1_l[:, F:].T  (host)
    with u_l[src_e] fetched by dma_gather (4 SWDGE queues).
  - Segment-sum by dst via per-tile selection-matrix matmuls into PSUM
    node-window accumulators (edges pre-sorted by (src<LO, dst-block)).
  - combine: h_l = relu(W2 @ [h; h_N] + b2) on TensorE, feature-major.
Host precomputes all index/layout arrays; the harness-visible entry point is
kernel(**inputs) -> np.ndarray [50000, 128] float32.
"""

import hashlib
import numpy as np
import ml_dtypes

import concourse.bass as bass
import concourse.bacc as bacc
import concourse.tile as tile
import concourse.mybir as mybir
from concourse.bass_utils import run_bass_kernel_spmd

BF16 = ml_dtypes.bfloat16
F32 = np.float32

N_NODES = 50000
N_EDGES = 800000
F0 = 32           # input feats (num gate types)
H = 128           # hidden dim
NC = 8            # cores
NPC = N_NODES // NC      # 6250 nodes per core
NB = 49                  # dst 128-blocks per core (49*128 = 6272)
NBW = NB * 128           # padded own-node count
LO = 25088               # src < LO uses the lo table view (int16 index limit)
TILE = 128
CALL_TILES = 24          # tiles per dma_gather call (3072 rows)
PRELU_ALPHA = 0.01

PROFILE = False          # set True (e.g. from test.py) to capture HW timing
LAST_EXEC_NS = None

_cache = {}


# ----------------------------------------------------------------- host prep

def _schedule_and_arrays(gate_type, edge_src, edge_dst, edge_w,
                         emb, W1_0, W2_0, b2_0, W1_rest, W2_rest, b2_rest):
    src_all = np.asarray(edge_src).astype(np.int64)
    dst_all = np.asarray(edge_dst).astype(np.int64)
    gt_all = np.asarray(gate_type).astype(np.int64)
    w_all = np.asarray(edge_w).astype(np.float32)
    core_of = dst_all // NPC

    # per-core edge sets
    pc = []
    for c in range(NC):
        m = core_of == c
        s = src_all[m]
        dl = dst_all[m] - c * NPC
        pc.append((s, dl, w_all[m]))

    # counts per (core, pass, block);  pass 0 = src<LO, pass 1 = src>=LO
    cnt = np.zeros((NC, 2, NB), np.int64)
    for c in range(NC):
        s, dl, _ = pc[c]
        key = (s >= LO).astype(np.int64) * NB + dl // TILE
        bc = np.bincount(key, minlength=2 * NB)
        cnt[c] = bc.reshape(2, NB)

    ntiles = np.ceil(cnt.max(axis=0) / TILE).astype(np.int64)  # [2, NB]
    ntiles[0] = np.maximum(ntiles[0], 1)   # every block gets a pass-0 session

    # global tile list: pass 0 blocks 0..NB-1, then pass 1
    tiles = []          # (pass, block)
    sess_start = []
    sess_end = []
    tile_base = {}      # (p, b) -> first tile index
    for p in (0, 1):
        for b in range(NB):
            nt = int(ntiles[p][b])
            if nt == 0:
                continue
            tile_base[(p, b)] = len(tiles)
            for j in range(nt):
                tiles.append((p, b))
                sess_start.append(j == 0)
                sess_end.append(j == nt - 1)
    NT = len(tiles)
    pass0_tiles = int(ntiles[0].sum())

    # gather calls: chunks of CALL_TILES within each pass
    calls = []          # (pass, t0, t1)
    for p, lo_t, hi_t in ((0, 0, pass0_tiles), (1, pass0_tiles, NT)):
        t = lo_t
        while t < hi_t:
            t1 = min(t + CALL_TILES, hi_t)
            calls.append((p, t, t1))
            t = t1

    # per-core slot assignment + host arrays
    group_base = np.zeros(2 * NB, np.int64)
    for p in (0, 1):
        for b in range(NB):
            if (p, b) in tile_base:
                group_base[p * NB + b] = tile_base[(p, b)] * TILE

    W1w = [np.asarray(W1_0)[:, F0:F0 + 3],
           np.asarray(W1_rest)[0][:, H:H + 3],
           np.asarray(W1_rest)[1][:, H:H + 3]]
    h0_full = np.asarray(emb)[gt_all]          # [N, F0]

    per_core = []
    for c in range(NC):
        s, dl, w = pc[c]
        key = (s >= LO).astype(np.int64) * NB + dl // TILE
        order = np.argsort(key, kind="stable")
        ks = key[order]
        first = np.zeros(2 * NB, np.int64)
        np.cumsum(np.bincount(ks, minlength=2 * NB)[:-1], out=first[1:])
        rank = np.arange(len(ks)) - first[ks]
        slot = group_base[ks] + rank           # global slot per sorted edge

        so, dlo, wo = s[order], dl[order], w[order]
        idx12 = np.zeros(NT * TILE, np.int16)
        idx12[slot] = np.where(so < LO, so, so - LO).astype(np.int16)
        idx0 = np.zeros(NT * TILE, np.int16)
        idx0[slot] = gt_all[so].astype(np.int16)
        dstloc = np.full(NT * TILE, 200.0, np.float32)
        dstloc[slot] = (dlo % TILE).astype(np.float32)

        vs = []
        for l in range(3):
            vfull = np.zeros((NT * TILE, H), np.float32)
            vfull[slot] = wo @ W1w[l].T
            vs.append(np.ascontiguousarray(
                vfull.reshape(NT, TILE, H).transpose(1, 0, 2)).astype(BF16))

        def wrap(a):
            outs = []
            for (_, t0, t1) in calls:
                seg = a[t0 * TILE:t1 * TILE].reshape(-1, 16).T  # [16, ct*8]
                outs.append(np.tile(seg, (8, 1)))
            return np.ascontiguousarray(np.concatenate(outs, axis=1))

        cnts = np.bincount(dl, minlength=NBW).astype(np.float32)
        inv_cnt = (1.0 / np.maximum(cnts, 1.0)).reshape(NB, TILE).T  # [128, NB]

        h0T = np.zeros((F0, NBW), np.float32)
        h0T[:, :NPC] = h0_full[c * NPC:(c + 1) * NPC].T

        per_core.append({
            "idx12": wrap(idx12),
            "idx0": wrap(idx0),
            "dstloc": np.ascontiguousarray(
                dstloc.reshape(NT, TILE).T).astype(BF16),
            "v0": vs[0], "v1": vs[1], "v2": vs[2],
            "inv_cnt": np.ascontiguousarray(inv_cnt).astype(F32),
            "h0T": h0T.astype(BF16),
        })

    # shared weights
    table0 = (np.asarray(emb) @ np.asarray(W1_0)[:, :F0].T).astype(BF16)  # [32,128]
    w1ht = np.stack([np.asarray(W1_rest)[0][:, :H].T,
                     np.asarray(W1_rest)[1][:, :H].T]).astype(BF16)
    w2at0 = np.asarray(W2_0)[:, :F0].T.astype(BF16)        # [32, 128]
    w2bt0 = np.asarray(W2_0)[:, F0:].T.astype(BF16)        # [128, 128]
    w2at12 = np.stack([np.asarray(W2_rest)[0][:, :H].T,
                       np.asarray(W2_rest)[1][:, :H].T]).astype(BF16)
    w2bt12 = np.stack([np.asarray(W2_rest)[0][:, H:].T,
                       np.asarray(W2_rest)[1][:, H:].T]).astype(BF16)
    b2t = np.stack([np.asarray(b2_0),
                    np.asarray(b2_rest)[0],
                    np.asarray(b2_rest)[1]]).T.astype(F32)  # [128, 3]
    iota_row = np.tile(np.arange(TILE, dtype=np.float32), (TILE, 1)).astype(BF16)
    ident = np.eye(TILE, dtype=np.float32).astype(BF16)

    shared = {
        "table0": table0, "w1ht": w1ht,
        "w2at0": w2at0, "w2bt0": w2bt0,
        "w2at12": w2at12, "w2bt12": w2bt12,
        "b2t": b2t, "iota": iota_row, "ident": ident,
    }
    for m in per_core:
        m.update(shared)

    sched = {
        "NT": NT, "tiles": tiles, "sess_start": sess_start,
        "sess_end": sess_end, "calls": calls,
    }
    return sched, per_core


# ------------------------------------------------------------------- codegen

def _ap3(ap2, inner):
    """[P, K] AP -> [P, K, inner] AP broadcast along a new 0-stride inner."""
    return bass.AP(ap2.tensor, ap2.offset,
                   [list(ap2.ap[0]), list(ap2.ap[1]), [0, inner]])


def _emit_prologue(env):
    nc, pools, P, dt = env["nc"], env["pools"], env["P"], env["dt"]
    NT = env["sched"]["NT"]
    constp = pools["const"]

    def load_const(name, shape, dtyp):
        t_ = constp.tile(shape, dtyp, tag=name)
        nc.sync.dma_start(t_[:], P[name][:])
        return t_

    env["iota_sb"] = load_const("iota", [128, 128], dt.bfloat16)
    env["ident_sb"] = load_const("ident", [128, 128], dt.bfloat16)
    env["dstloc_sb"] = load_const("dstloc", [128, NT], dt.bfloat16)
    env["inv_sb"] = load_const("inv_cnt", [128, NB], dt.float32)
    env["b2_sb"] = load_const("b2t", [H, 3], dt.float32)
    env["w2at0_sb"] = load_const("w2at0", [F0, H], dt.bfloat16)
    env["w2bt0_sb"] = load_const("w2bt0", [H, H], dt.bfloat16)
    for nm in ("w1ht", "w2at12", "w2bt12"):
        t_ = constp.tile([H, 2, H], dt.bfloat16, tag=nm)
        nc.sync.dma_start(t_[:], P[nm].ap().rearrange("a k m -> k a m"))
        env[nm + "_sb"] = t_
    idx12_sb = pools["idxr"].tile([128, NT * 8], dt.int16)
    nc.sync.dma_start(idx12_sb[:], P["idx12"][:])
    env["idx12_sb"] = idx12_sb
    h0T_sb = constp.tile([F0, NBW], dt.bfloat16, tag="h0T")
    nc.sync.dma_start(h0T_sb[:], P["h0T"][:])
    env["h_prevT"] = h0T_sb
    env["s_acc"] = pools["acc"].tile([128, NB, H], dt.float32, name="s_acc")
    # idx column offsets per call (wrapped layout)
    call_cols = []
    off = 0
    for (_, t0, t1) in env["sched"]["calls"]:
        ct = t1 - t0
        call_cols.append(off)
        off += ct * 8
    env["call_cols"] = call_cols


def _emit_u_allgather(env, l):
    nc, pools, dt = env["nc"], env["pools"], env["dt"]
    w1 = env["w1ht_sb"][:, l - 1, :]
    h_prevT = env["h_prevT"]
    uT = pools["scr"].tile([H, NBW], dt.bfloat16, tag="scr")
    for ck0 in range(0, NBW, 512):
        ck = min(512, NBW - ck0)
        pu = pools["p512"].tile([128, 512], dt.float32)
        nc.tensor.matmul(pu[:, :ck], w1, h_prevT[:, ck0:ck0 + ck],
                         start=True, stop=True)
        nc.scalar.activation(uT[:, ck0:ck0 + ck], pu[:, :ck],
                             mybir.ActivationFunctionType.Copy)
    u_nm = pools["unm"].tile([128, NB, H], dt.bfloat16)
    for b in range(NB):
        ptr = pools["ptr"].tile([128, 128], dt.bfloat16)
        nc.tensor.transpose(ptr[:], uT[:, b * 128:(b + 1) * 128], env["ident_sb"][:])
        nc.vector.tensor_copy(u_nm[:, b, :], ptr[:])
        nc.sync.dma_start(env["u_own"][b * 128:(b + 1) * 128, :], u_nm[:, b, :])
    nc.gpsimd.collective_compute(
        "AllGather", mybir.AluOpType.bypass,
        replica_groups=[list(range(NC))],
        ins=[env["u_own"][0:NPC, :].opt()],
        outs=[env["u_table"][0:N_NODES, :].opt()],
    )


def _emit_edge_tile(env, l, gt, tt_ap, S_ap, state):
    """Selection matmul + session bookkeeping for one 128-edge tile."""
    nc, pools, dt = env["nc"], env["pools"], env["dt"]
    sched = env["sched"]
    pss_t, b = sched["tiles"][gt]
    if sched["sess_start"][gt]:
        state["psum"] = pools["psw"].tile([128, 128], dt.float32, name="psw")
    nc.tensor.matmul(state["psum"][:], S_ap, tt_ap,
                     start=bool(sched["sess_start"][gt]),
                     stop=bool(sched["sess_end"][gt]))
    if sched["sess_end"][gt]:
        s_acc = env["s_acc"]
        if pss_t == 0:
            nc.vector.tensor_copy(s_acc[:, b, :], state["psum"][:])
        else:
            nc.vector.tensor_tensor(
                out=s_acc[:, b, :], in0=s_acc[:, b, :],
                in1=state["psum"][:], op=mybir.AluOpType.add)


def _emit_call(env, l, ci, state):
    nc, pools, dt = env["nc"], env["pools"], env["dt"]
    pss, t0, t1 = env["sched"]["calls"][ci]
    ct = t1 - t0
    coff = env["call_cols"][ci]
    if l == 0:
        ixt = pools["idxs"].tile([128, CALL_TILES * 8], dt.int16, tag="ix")
        nc.sync.dma_start(ixt[:, :ct * 8], env["P"]["idx0"][:, coff:coff + ct * 8])
        idx_ap = ixt[:, :ct * 8]
        view = env["P"]["table0"][:]
    else:
        idx_ap = env["idx12_sb"][:, coff:coff + ct * 8]
        u_table = env["u_table"]
        view = u_table[0:LO, :] if pss == 0 else u_table[LO:50176, :]

    g = pools["g"].tile([128, CALL_TILES, H], dt.bfloat16, tag="g")
    nc.gpsimd.dma_gather(
        out_ap=g[:, :ct, :], in_ap=view, idxs_ap=idx_ap,
        num_idxs=ct * 128, num_idxs_reg=ct * 128, elem_size=H,
        single_packet=False, queue_num=ci % 4,
    )
    vsl = pools["vsl"].tile([128, CALL_TILES, H], dt.bfloat16, tag="v")
    nc.sync.dma_start(vsl[:, :ct, :], env["P"][f"v{l}"][:, t0:t1, :])

    for j4 in range(0, ct, 4):
        nj = min(4, ct - j4)
        y = pools["y"].tile([128, 4, H], dt.bfloat16, tag="y")
        nc.vector.tensor_tensor(
            out=y[:, :nj, :], in0=g[:, j4:j4 + nj, :],
            in1=vsl[:, j4:j4 + nj, :], op=mybir.AluOpType.add)
        tt = pools["t"].tile([128, 4, H], dt.bfloat16, tag="t")
        nc.scalar.activation(tt[:, :nj, :], y[:, :nj, :],
                             mybir.ActivationFunctionType.Prelu,
                             alpha=PRELU_ALPHA)
        S4 = pools["S"].tile([128, 4, 128], dt.bfloat16, tag="S")
        gt0 = t0 + j4
        iap = env["iota_sb"][:, :]
        in0 = bass.AP(iap.tensor, iap.offset,
                      [list(iap.ap[0]), [0, nj], list(iap.ap[1])])
        dap = env["dstloc_sb"][:, gt0:gt0 + nj]
        in1 = bass.AP(dap.tensor, dap.offset,
                      [list(dap.ap[0]), list(dap.ap[1]), [0, 128]])
        nc.vector.tensor_tensor(out=S4[:, :nj, :], in0=in0, in1=in1,
                                op=mybir.AluOpType.is_equal)
        for jj in range(nj):
            _emit_edge_tile(env, l, gt0 + jj, tt[:, jj, :], S4[:, jj, :], state)


def _emit_epilogue(env, l):
    nc, pools, dt = env["nc"], env["pools"], env["dt"]
    # h_N = s * inv_cnt, transposed to feature-major
    hNT = pools["scr"].tile([H, NBW], dt.bfloat16, tag="scr")
    for b in range(NB):
        hn = pools["small"].tile([128, 128], dt.bfloat16, tag="hn")
        nc.vector.tensor_tensor(
            out=hn[:], in0=env["s_acc"][:, b, :],
            in1=env["inv_sb"][:, b:b + 1].to_broadcast([128, 128]),
            op=mybir.AluOpType.mult)
        ptr = pools["ptr"].tile([128, 128], dt.bfloat16)
        nc.tensor.transpose(ptr[:], hn[:], env["ident_sb"][:])
        nc.vector.tensor_copy(hNT[:, b * 128:(b + 1) * 128], ptr[:])

    # combine: h_out = relu(W2a @ h_prev + W2b @ h_N + b2)
    if l == 0:
        w2a, w2b = env["w2at0_sb"][:], env["w2bt0_sb"][:]
    else:
        w2a = env["w2at12_sb"][:, l - 1, :]
        w2b = env["w2bt12_sb"][:, l - 1, :]
    bias = env["b2_sb"][:, l:l + 1]
    h_outT = None
    if l < 2:
        h_outT = pools["h"].tile([H, NBW], dt.bfloat16, tag="h")
    for ck0 in range(0, NBW, 512):
        ck = min(512, NBW - ck0)
        pc_ = pools["p512"].tile([128, 512], dt.float32)
        nc.tensor.matmul(pc_[:, :ck], w2a, env["h_prevT"][:, ck0:ck0 + ck],
                         start=True, stop=False)
        nc.tensor.matmul(pc_[:, :ck], w2b, hNT[:, ck0:ck0 + ck],
                         start=False, stop=True)
        if l < 2:
            nc.scalar.activation(h_outT[:, ck0:ck0 + ck], pc_[:, :ck],
                                 mybir.ActivationFunctionType.Relu, bias=bias)
        else:
            oc = pools["oc"].tile([H, 512], dt.float32, tag="oc")
            nc.scalar.activation(oc[:, :ck], pc_[:, :ck],
                                 mybir.ActivationFunctionType.Relu, bias=bias)
            nc.sync.dma_start(env["out_ext"][:, ck0:ck0 + ck], oc[:, :ck])
    if l < 2:
        env["h_prevT"] = h_outT


def _emit_program(env):
    _emit_prologue(env)
    ncalls = len(env["sched"]["calls"])
    for l in range(3):
        if l > 0:
            _emit_u_allgather(env, l)
        state = {}
        for ci in range(ncalls):
            _emit_call(env, l, ci, state)
        _emit_epilogue(env, l)


def _build_nc(sched):
    NT = sched["NT"]
    dt = mybir.dt

    nc = bacc.Bacc("TRN2", target_bir_lowering=False, debug=False,
                   num_devices=NC, num_swdge_queues=4,
                   dynamic_dma_scratch_size=32768)

    P = {}
    P["idx12"] = nc.dram_tensor("idx12", [128, NT * 8], dt.int16, kind="ExternalInput")
    P["idx0"] = nc.dram_tensor("idx0", [128, NT * 8], dt.int16, kind="ExternalInput")
    P["dstloc"] = nc.dram_tensor("dstloc", [128, NT], dt.bfloat16, kind="ExternalInput")
    for l in range(3):
        P[f"v{l}"] = nc.dram_tensor(f"v{l}", [128, NT, H], dt.bfloat16, kind="ExternalInput")
    P["inv_cnt"] = nc.dram_tensor("inv_cnt", [128, NB], dt.float32, kind="ExternalInput")
    P["h0T"] = nc.dram_tensor("h0T", [F0, NBW], dt.bfloat16, kind="ExternalInput")
    P["table0"] = nc.dram_tensor("table0", [F0, H], dt.bfloat16, kind="ExternalInput")
    P["w1ht"] = nc.dram_tensor("w1ht", [2, H, H], dt.bfloat16, kind="ExternalInput")
    P["w2at0"] = nc.dram_tensor("w2at0", [F0, H], dt.bfloat16, kind="ExternalInput")
    P["w2bt0"] = nc.dram_tensor("w2bt0", [H, H], dt.bfloat16, kind="ExternalInput")
    P["w2at12"] = nc.dram_tensor("w2at12", [2, H, H], dt.bfloat16, kind="ExternalInput")
    P["w2bt12"] = nc.dram_tensor("w2bt12", [2, H, H], dt.bfloat16, kind="ExternalInput")
    P["b2t"] = nc.dram_tensor("b2t", [H, 3], dt.float32, kind="ExternalInput")
    P["iota"] = nc.dram_tensor("iota", [128, 128], dt.bfloat16, kind="ExternalInput")
    P["ident"] = nc.dram_tensor("ident", [128, 128], dt.bfloat16, kind="ExternalInput")

    out_ext = nc.dram_tensor("out", [H, NBW], dt.float32, kind="ExternalOutput")
    u_own = nc.dram_tensor("u_own", [NBW, H], dt.bfloat16)
    u_table = nc.dram_tensor("u_table", [50176, H], dt.bfloat16, addr_space="Shared")

    from contextlib import ExitStack
    with tile.TileContext(nc) as tc, ExitStack() as ctx:
        pools = {}
        for nm, bufs, space in [
            ("const", 1, "SBUF"), ("idxr", 1, "SBUF"), ("idxs", 4, "SBUF"),
            ("g", 5, "SBUF"), ("vsl", 3, "SBUF"), ("y", 4, "SBUF"),
            ("t", 4, "SBUF"), ("S", 6, "SBUF"), ("acc", 1, "SBUF"),
            ("h", 2, "SBUF"), ("scr", 2, "SBUF"), ("unm", 1, "SBUF"),
            ("small", 4, "SBUF"), ("oc", 3, "SBUF"),
            ("psw", 2, "PSUM"), ("ptr", 2, "PSUM"), ("p512", 2, "PSUM"),
        ]:
            pools[nm] = ctx.enter_context(tc.tile_pool(name=nm, bufs=bufs, space=space))
        env = dict(nc=nc, tc=tc, pools=pools, P=P, out_ext=out_ext,
                   u_own=u_own, u_table=u_table, sched=sched, dt=dt)
        _emit_program(env)

    nc.compile()
    return nc


# --------------------------------------------------------------------- entry

def kernel(gate_type, edge_src, edge_dst, edge_w, emb, W1_0, W2_0, b2_0,
           W1_rest, W2_rest, b2_rest):
    global LAST_EXEC_NS
    key = hashlib.sha1(
        np.ascontiguousarray(np.asarray(edge_dst, dtype=np.int64)).tobytes()
        + np.ascontiguousarray(np.asarray(edge_src, dtype=np.int64)).tobytes()
    ).hexdigest()

    sched, per_core = _schedule_and_arrays(
        gate_type, edge_src, edge_dst, edge_w, emb, W1_0, W2_0, b2_0,
        W1_rest, W2_rest, b2_rest)

    if key in _cache and _cache[key][1]["NT"] == sched["NT"]:
        nc = _cache[key][0]
    else:
        nc = _build_nc(sched)
        _cache.clear()
        _cache[key] = (nc, sched)

    res = run_bass_kernel_spmd(nc, per_core, core_ids=list(range(NC)),
                               trace=PROFILE)
    LAST_EXEC_NS = res.exec_time_ns

    out = np.empty((N_NODES, H), np.float32)
    for c in range(NC):
        out[c * NPC:(c + 1) * NPC] = res.results[c]["out"][:, :NPC].T
    return out


# revision 17
# speedup vs baseline: 1.0607x; 1.0607x over previous
"""Trainium2 Bass kernel for nn_ActorCritic (3-layer edge-GNN, qconv stack).

Strategy (8 NeuronCores):
  - Nodes sharded 8 ways by dst: core c owns nodes [c*6250, (c+1)*6250).
  - Edges assigned to the core owning their dst; segment-sum is core-local.
  - Per layer l: u_l = h_{l-1} @ W1_l[:, :F].T is computed per-core on own
    nodes and AllGathered into a DRAM table; per-edge work is
      t_e = leaky_relu(u_l[src_e] + v_e),   v_e = w_e @ W1_l[:, F:].T  (host)
    with u_l[src_e] fetched by dma_gather (4 SWDGE queues).
  - Segment-sum by dst via per-tile selection-matrix matmuls into PSUM
    node-window accumulators (edges pre-sorted by (src<LO, dst-block)).
  - combine: h_l = relu(W2 @ [h; h_N] + b2) on TensorE, feature-major.
Host precomputes all index/layout arrays; the harness-visible entry point is
kernel(**inputs) -> np.ndarray [50000, 128] float32.
"""

import hashlib
import numpy as np
import ml_dtypes

import concourse.bass as bass
import concourse.bacc as bacc
import concourse.tile as tile
import concourse.mybir as mybir
from concourse.bass_utils import run_bass_kernel_spmd

BF16 = ml_dtypes.bfloat16
F32 = np.float32

N_NODES = 50000
N_EDGES = 800000
F0 = 32           # input feats (num gate types)
H = 128           # hidden dim
NC = 8            # cores
NPC = N_NODES // NC      # 6250 nodes per core
NB = 49                  # dst 128-blocks per core (49*128 = 6272)
NBW = NB * 128           # padded own-node count
LO = 25088               # src < LO uses the lo table view (int16 index limit)
TILE = 128
CALL_TILES = 24          # tiles per dma_gather call (3072 rows)
PRELU_ALPHA = 0.01

PROFILE = False          # set True (e.g. from test.py) to capture HW timing
LAST_EXEC_NS = None

_cache = {}


# ----------------------------------------------------------------- host prep

def _schedule_and_arrays(gate_type, edge_src, edge_dst, edge_w,
                         emb, W1_0, W2_0, b2_0, W1_rest, W2_rest, b2_rest):
    src_all = np.asarray(edge_src).astype(np.int64)
    dst_all = np.asarray(edge_dst).astype(np.int64)
    gt_all = np.asarray(gate_type).astype(np.int64)
    w_all = np.asarray(edge_w).astype(np.float32)
    core_of = dst_all // NPC

    # per-core edge sets
    pc = []
    for c in range(NC):
        m = core_of == c
        s = src_all[m]
        dl = dst_all[m] - c * NPC
        pc.append((s, dl, w_all[m]))

    # counts per (core, pass, block);  pass 0 = src<LO, pass 1 = src>=LO
    cnt = np.zeros((NC, 2, NB), np.int64)
    for c in range(NC):
        s, dl, _ = pc[c]
        key = (s >= LO).astype(np.int64) * NB + dl // TILE
        bc = np.bincount(key, minlength=2 * NB)
        cnt[c] = bc.reshape(2, NB)

    ntiles = np.ceil(cnt.max(axis=0) / TILE).astype(np.int64)  # [2, NB]
    ntiles[0] = np.maximum(ntiles[0], 1)   # every block gets a pass-0 session

    # global tile list: pass 0 blocks 0..NB-1, then pass 1
    tiles = []          # (pass, block)
    sess_start = []
    sess_end = []
    tile_base = {}      # (p, b) -> first tile index
    for p in (0, 1):
        for b in range(NB):
            nt = int(ntiles[p][b])
            if nt == 0:
                continue
            tile_base[(p, b)] = len(tiles)
            for j in range(nt):
                tiles.append((p, b))
                sess_start.append(j == 0)
                sess_end.append(j == nt - 1)
    NT = len(tiles)
    pass0_tiles = int(ntiles[0].sum())

    # gather calls: chunks of CALL_TILES within each pass
    calls = []          # (pass, t0, t1)
    for p, lo_t, hi_t in ((0, 0, pass0_tiles), (1, pass0_tiles, NT)):
        t = lo_t
        while t < hi_t:
            t1 = min(t + CALL_TILES, hi_t)
            calls.append((p, t, t1))
            t = t1

    # per-core slot assignment + host arrays
    group_base = np.zeros(2 * NB, np.int64)
    for p in (0, 1):
        for b in range(NB):
            if (p, b) in tile_base:
                group_base[p * NB + b] = tile_base[(p, b)] * TILE

    W1w = [np.asarray(W1_0)[:, F0:F0 + 3],
           np.asarray(W1_rest)[0][:, H:H + 3],
           np.asarray(W1_rest)[1][:, H:H + 3]]
    h0_full = np.asarray(emb)[gt_all]          # [N, F0]

    per_core = []
    for c in range(NC):
        s, dl, w = pc[c]
        key = (s >= LO).astype(np.int64) * NB + dl // TILE
        order = np.argsort(key, kind="stable")
        ks = key[order]
        first = np.zeros(2 * NB, np.int64)
        np.cumsum(np.bincount(ks, minlength=2 * NB)[:-1], out=first[1:])
        rank = np.arange(len(ks)) - first[ks]
        slot = group_base[ks] + rank           # global slot per sorted edge

        so, dlo, wo = s[order], dl[order], w[order]
        idx12 = np.zeros(NT * TILE, np.int16)
        idx12[slot] = np.where(so < LO, so, so - LO).astype(np.int16)
        idx0 = np.zeros(NT * TILE, np.int16)
        idx0[slot] = gt_all[so].astype(np.int16)
        dstloc = np.full(NT * TILE, 200.0, np.float32)
        dstloc[slot] = (dlo % TILE).astype(np.float32)

        vs = []
        for l in range(3):
            vfull = np.zeros((NT * TILE, H), np.float32)
            vfull[slot] = wo @ W1w[l].T
            vs.append(np.ascontiguousarray(
                vfull.reshape(NT, TILE, H).transpose(1, 0, 2)).astype(BF16))

        def wrap(a):
            outs = []
            for (_, t0, t1) in calls:
                seg = a[t0 * TILE:t1 * TILE].reshape(-1, 16).T  # [16, ct*8]
                outs.append(np.tile(seg, (8, 1)))
            return np.ascontiguousarray(np.concatenate(outs, axis=1))

        cnts = np.bincount(dl, minlength=NBW).astype(np.float32)
        inv_cnt = (1.0 / np.maximum(cnts, 1.0)).reshape(NB, TILE).T  # [128, NB]

        h0T = np.zeros((F0, NBW), np.float32)
        h0T[:, :NPC] = h0_full[c * NPC:(c + 1) * NPC].T

        per_core.append({
            "idx12": wrap(idx12),
            "idx0": wrap(idx0),
            "dstloc": np.ascontiguousarray(
                dstloc.reshape(NT, TILE).T).astype(BF16),
            "v0": vs[0], "v1": vs[1], "v2": vs[2],
            "inv_cnt": np.ascontiguousarray(inv_cnt).astype(F32),
            "h0T": h0T.astype(BF16),
        })

    # shared weights
    table0 = (np.asarray(emb) @ np.asarray(W1_0)[:, :F0].T).astype(BF16)  # [32,128]
    w1ht = np.stack([np.asarray(W1_rest)[0][:, :H].T,
                     np.asarray(W1_rest)[1][:, :H].T]).astype(BF16)
    w2at0 = np.asarray(W2_0)[:, :F0].T.astype(BF16)        # [32, 128]
    w2bt0 = np.asarray(W2_0)[:, F0:].T.astype(BF16)        # [128, 128]
    w2at12 = np.stack([np.asarray(W2_rest)[0][:, :H].T,
                       np.asarray(W2_rest)[1][:, :H].T]).astype(BF16)
    w2bt12 = np.stack([np.asarray(W2_rest)[0][:, H:].T,
                       np.asarray(W2_rest)[1][:, H:].T]).astype(BF16)
    b2t = np.stack([np.asarray(b2_0),
                    np.asarray(b2_rest)[0],
                    np.asarray(b2_rest)[1]]).T.astype(F32)  # [128, 3]
    iota_row = np.tile(np.arange(TILE, dtype=np.float32), (TILE, 1)).astype(BF16)
    ident = np.eye(TILE, dtype=np.float32).astype(BF16)

    shared = {
        "table0": table0, "w1ht": w1ht,
        "w2at0": w2at0, "w2bt0": w2bt0,
        "w2at12": w2at12, "w2bt12": w2bt12,
        "b2t": b2t, "iota": iota_row, "ident": ident,
    }
    for m in per_core:
        m.update(shared)

    sched = {
        "NT": NT, "tiles": tiles, "sess_start": sess_start,
        "sess_end": sess_end, "calls": calls,
    }
    return sched, per_core


# ------------------------------------------------------------------- codegen

def _ap3(ap2, inner):
    """[P, K] AP -> [P, K, inner] AP broadcast along a new 0-stride inner."""
    return bass.AP(ap2.tensor, ap2.offset,
                   [list(ap2.ap[0]), list(ap2.ap[1]), [0, inner]])


def _emit_prologue(env):
    nc, pools, P, dt = env["nc"], env["pools"], env["P"], env["dt"]
    NT = env["sched"]["NT"]
    constp = pools["const"]

    def load_const(name, shape, dtyp):
        t_ = constp.tile(shape, dtyp, tag=name)
        nc.sync.dma_start(t_[:], P[name][:])
        return t_

    env["iota_sb"] = load_const("iota", [128, 128], dt.bfloat16)
    env["ident_sb"] = load_const("ident", [128, 128], dt.bfloat16)
    env["dstloc_sb"] = load_const("dstloc", [128, NT], dt.bfloat16)
    env["inv_sb"] = load_const("inv_cnt", [128, NB], dt.float32)
    env["b2_sb"] = load_const("b2t", [H, 3], dt.float32)
    env["w2at0_sb"] = load_const("w2at0", [F0, H], dt.bfloat16)
    env["w2bt0_sb"] = load_const("w2bt0", [H, H], dt.bfloat16)
    for nm in ("w1ht", "w2at12", "w2bt12"):
        t_ = constp.tile([H, 2, H], dt.bfloat16, tag=nm)
        nc.sync.dma_start(t_[:], P[nm].ap().rearrange("a k m -> k a m"))
        env[nm + "_sb"] = t_
    idx12_sb = pools["idxr"].tile([128, NT * 8], dt.int16)
    nc.sync.dma_start(idx12_sb[:], P["idx12"][:])
    env["idx12_sb"] = idx12_sb
    h0T_sb = constp.tile([F0, NBW], dt.bfloat16, tag="h0T")
    nc.sync.dma_start(h0T_sb[:], P["h0T"][:])
    env["h_prevT"] = h0T_sb
    env["s_acc"] = pools["acc"].tile([128, NB, H], dt.float32, name="s_acc")
    # idx column offsets per call (wrapped layout)
    call_cols = []
    off = 0
    for (_, t0, t1) in env["sched"]["calls"]:
        ct = t1 - t0
        call_cols.append(off)
        off += ct * 8
    env["call_cols"] = call_cols


def _emit_allgather(env):
    nc = env["nc"]
    nc.gpsimd.collective_compute(
        "AllGather", mybir.AluOpType.bypass,
        replica_groups=[list(range(NC))],
        ins=[env["u_own"][0:NPC, :].opt()],
        outs=[env["u_table"][0:N_NODES, :].opt()],
    )


def _emit_edge_tile(env, l, gt, tt_ap, S_ap, state):
    """Selection matmul + session bookkeeping for one 128-edge tile."""
    nc, pools, dt = env["nc"], env["pools"], env["dt"]
    sched = env["sched"]
    pss_t, b = sched["tiles"][gt]
    if sched["sess_start"][gt]:
        state["psum"] = pools["psw"].tile([128, 128], dt.float32, name="psw")
    nc.tensor.matmul(state["psum"][:], S_ap, tt_ap,
                     start=bool(sched["sess_start"][gt]),
                     stop=bool(sched["sess_end"][gt]))
    if sched["sess_end"][gt]:
        s_acc = env["s_acc"]
        if pss_t == 0:
            nc.vector.tensor_copy(s_acc[:, b, :], state["psum"][:])
        else:
            nc.vector.tensor_tensor(
                out=s_acc[:, b, :], in0=s_acc[:, b, :],
                in1=state["psum"][:], op=mybir.AluOpType.add)


def _emit_call(env, l, ci, state):
    nc, pools, dt = env["nc"], env["pools"], env["dt"]
    pss, t0, t1 = env["sched"]["calls"][ci]
    ct = t1 - t0
    coff = env["call_cols"][ci]
    if l == 0:
        ixt = pools["idxs"].tile([128, CALL_TILES * 8], dt.int16, tag="ix")
        nc.sync.dma_start(ixt[:, :ct * 8], env["P"]["idx0"][:, coff:coff + ct * 8])
        idx_ap = ixt[:, :ct * 8]
        view = env["P"]["table0"][:]
    else:
        idx_ap = env["idx12_sb"][:, coff:coff + ct * 8]
        u_table = env["u_table"]
        view = u_table[0:LO, :] if pss == 0 else u_table[LO:50176, :]

    g = pools["g"].tile([128, CALL_TILES, H], dt.bfloat16, tag="g")
    nc.gpsimd.dma_gather(
        out_ap=g[:, :ct, :], in_ap=view, idxs_ap=idx_ap,
        num_idxs=ct * 128, num_idxs_reg=ct * 128, elem_size=H,
        single_packet=False, queue_num=ci % 4,
    )
    vsl = pools["vsl"].tile([128, CALL_TILES, H], dt.bfloat16, tag="v")
    nc.sync.dma_start(vsl[:, :ct, :], env["P"][f"v{l}"][:, t0:t1, :])

    import os
    if os.environ.get("GATHER_ONLY"):
        if ci == len(env["sched"]["calls"]) - 1:
            nc.vector.tensor_copy(env["s_acc"][:, 0, :], g[:, 0, :])
        return
    for j4 in range(0, ct, 4):
        nj = min(4, ct - j4)
        y = pools["y"].tile([128, 4, H], dt.bfloat16, tag="y")
        nc.vector.tensor_tensor(
            out=y[:, :nj, :], in0=g[:, j4:j4 + nj, :],
            in1=vsl[:, j4:j4 + nj, :], op=mybir.AluOpType.add)
        tt = pools["t"].tile([128, 4, H], dt.bfloat16, tag="t")
        nc.scalar.activation(tt[:, :nj, :], y[:, :nj, :],
                             mybir.ActivationFunctionType.Prelu,
                             alpha=PRELU_ALPHA)
        S4 = pools["S"].tile([128, 4, 128], dt.bfloat16, tag="S")
        gt0 = t0 + j4
        iap = env["iota_sb"][:, :]
        in0 = bass.AP(iap.tensor, iap.offset,
                      [list(iap.ap[0]), [0, nj], list(iap.ap[1])])
        dap = env["dstloc_sb"][:, gt0:gt0 + nj]
        in1 = bass.AP(dap.tensor, dap.offset,
                      [list(dap.ap[0]), list(dap.ap[1]), [0, 128]])
        nc.vector.tensor_tensor(out=S4[:, :nj, :], in0=in0, in1=in1,
                                op=mybir.AluOpType.is_equal)
        for jj in range(nj):
            _emit_edge_tile(env, l, gt0 + jj, tt[:, jj, :], S4[:, jj, :], state)


def _emit_chunk(env, l, k):
    """Epilogue for node chunk k (blocks 4k..): h_N scale+transpose, combine,
    and (l<2) next-layer u rows + store."""
    nc, pools, dt = env["nc"], env["pools"], env["dt"]
    b0 = 4 * k
    nb = min(4, NB - b0)
    ck0, ck = 512 * k, 128 * nb
    s_acc, inv = env["s_acc"], env["inv_sb"]

    hn = pools["small"].tile([128, 4, 128], dt.bfloat16, tag="hn")
    iap = inv[:, b0:b0 + nb]
    in1 = bass.AP(iap.tensor, iap.offset,
                  [list(iap.ap[0]), list(iap.ap[1]), [0, 128]])
    nc.vector.tensor_tensor(out=hn[:, :nb, :], in0=s_acc[:, b0:b0 + nb, :],
                            in1=in1, op=mybir.AluOpType.mult)
    hNT = pools["scr"].tile([H, 512], dt.bfloat16, tag="hNT")
    for j in range(nb):
        ptr = pools["ptr"].tile([128, 128], dt.bfloat16, tag="ptr", name="ptr")
        nc.tensor.transpose(ptr[:], hn[:, j, :], env["ident_sb"][:])
        nc.vector.tensor_copy(hNT[:, j * 128:(j + 1) * 128], ptr[:])

    if l == 0:
        w2a, w2b = env["w2at0_sb"][:], env["w2bt0_sb"][:]
    else:
        w2a = env["w2at12_sb"][:, l - 1, :]
        w2b = env["w2bt12_sb"][:, l - 1, :]
    bias = env["b2_sb"][:, l:l + 1]
    pc_ = pools["p512"].tile([128, 512], dt.float32, tag="p512", name="pc_")
    nc.tensor.matmul(pc_[:, :ck], w2a, env["h_prevT"][:, ck0:ck0 + ck],
                     start=True, stop=False)
    nc.tensor.matmul(pc_[:, :ck], w2b, hNT[:, :ck], start=False, stop=True)
    if l < 2:
        h_outT = env["h_outT"]
        nc.scalar.activation(h_outT[:, ck0:ck0 + ck], pc_[:, :ck],
                             mybir.ActivationFunctionType.Relu, bias=bias)
        # next-layer u rows for this chunk: u = W1h_{l+1} @ h_out
        pu = pools["p512"].tile([128, 512], dt.float32, tag="p512", name="pu")
        nc.tensor.matmul(pu[:, :ck], env["w1ht_sb"][:, l, :],
                         h_outT[:, ck0:ck0 + ck], start=True, stop=True)
        uTc = pools["scr"].tile([H, 512], dt.bfloat16, tag="uTc")
        nc.scalar.activation(uTc[:, :ck], pu[:, :ck],
                             mybir.ActivationFunctionType.Copy)
        u_nm = pools["unm"].tile([128, 4, H], dt.bfloat16, tag="unm")
        for j in range(nb):
            ptru = pools["ptr"].tile([128, 128], dt.bfloat16, tag="ptr", name="ptru")
            nc.tensor.transpose(ptru[:], uTc[:, j * 128:(j + 1) * 128],
                                env["ident_sb"][:])
            nc.vector.tensor_copy(u_nm[:, j, :], ptru[:])
            b = b0 + j
            nc.sync.dma_start(env["u_own"][b * 128:(b + 1) * 128, :],
                              u_nm[:, j, :])
    else:
        oc = pools["oc"].tile([H, 512], dt.float32, tag="oc")
        nc.scalar.activation(oc[:, :ck], pc_[:, :ck],
                             mybir.ActivationFunctionType.Relu, bias=bias)
        nc.sync.dma_start(env["out_ext"][:, ck0:ck0 + ck], oc[:, :ck])


def _emit_program(env):
    _emit_prologue(env)
    sched = env["sched"]
    ncalls = len(sched["calls"])
    NCH = (NB + 3) // 4

    # per-block final tile = end of its last session; chunk ready-tile
    final_tile = {}
    for gt, (p, b) in enumerate(sched["tiles"]):
        if sched["sess_end"][gt]:
            final_tile[b] = gt
    chunk_ready = [max(final_tile[b] for b in range(4 * k, min(4 * k + 4, NB)))
                   for k in range(NCH)]

    pools, dt = env["pools"], env["dt"]
    for l in range(3):
        if l < 2:
            env["h_outT"] = pools["h"].tile([H, NBW], dt.bfloat16, tag="h",
                                            name=f"h{l + 1}")
        nxt = 0
        state = {}
        for ci in range(ncalls):
            _emit_call(env, l, ci, state)
            t1 = sched["calls"][ci][2]
            while nxt < NCH and chunk_ready[nxt] < t1:
                _emit_chunk(env, l, nxt)
                nxt += 1
        while nxt < NCH:
            _emit_chunk(env, l, nxt)
            nxt += 1
        if l < 2:
            _emit_allgather(env)
            env["h_prevT"] = env["h_outT"]


def _build_nc(sched):
    NT = sched["NT"]
    dt = mybir.dt

    nc = bacc.Bacc("TRN2", target_bir_lowering=False, debug=False,
                   num_devices=NC, num_swdge_queues=4)

    P = {}
    P["idx12"] = nc.dram_tensor("idx12", [128, NT * 8], dt.int16, kind="ExternalInput")
    P["idx0"] = nc.dram_tensor("idx0", [128, NT * 8], dt.int16, kind="ExternalInput")
    P["dstloc"] = nc.dram_tensor("dstloc", [128, NT], dt.bfloat16, kind="ExternalInput")
    for l in range(3):
        P[f"v{l}"] = nc.dram_tensor(f"v{l}", [128, NT, H], dt.bfloat16, kind="ExternalInput")
    P["inv_cnt"] = nc.dram_tensor("inv_cnt", [128, NB], dt.float32, kind="ExternalInput")
    P["h0T"] = nc.dram_tensor("h0T", [F0, NBW], dt.bfloat16, kind="ExternalInput")
    P["table0"] = nc.dram_tensor("table0", [F0, H], dt.bfloat16, kind="ExternalInput")
    P["w1ht"] = nc.dram_tensor("w1ht", [2, H, H], dt.bfloat16, kind="ExternalInput")
    P["w2at0"] = nc.dram_tensor("w2at0", [F0, H], dt.bfloat16, kind="ExternalInput")
    P["w2bt0"] = nc.dram_tensor("w2bt0", [H, H], dt.bfloat16, kind="ExternalInput")
    P["w2at12"] = nc.dram_tensor("w2at12", [2, H, H], dt.bfloat16, kind="ExternalInput")
    P["w2bt12"] = nc.dram_tensor("w2bt12", [2, H, H], dt.bfloat16, kind="ExternalInput")
    P["b2t"] = nc.dram_tensor("b2t", [H, 3], dt.float32, kind="ExternalInput")
    P["iota"] = nc.dram_tensor("iota", [128, 128], dt.bfloat16, kind="ExternalInput")
    P["ident"] = nc.dram_tensor("ident", [128, 128], dt.bfloat16, kind="ExternalInput")

    out_ext = nc.dram_tensor("out", [H, NBW], dt.float32, kind="ExternalOutput")
    u_own = nc.dram_tensor("u_own", [NBW, H], dt.bfloat16)
    u_table = nc.dram_tensor("u_table", [50176, H], dt.bfloat16, addr_space="Shared")

    from contextlib import ExitStack
    with tile.TileContext(nc) as tc, ExitStack() as ctx:
        pools = {}
        for nm, bufs, space in [
            ("const", 1, "SBUF"), ("idxr", 1, "SBUF"), ("idxs", 4, "SBUF"),
            ("g", 8, "SBUF"), ("vsl", 4, "SBUF"), ("y", 6, "SBUF"),
            ("t", 6, "SBUF"), ("S", 8, "SBUF"), ("acc", 1, "SBUF"),
            ("h", 2, "SBUF"), ("scr", 2, "SBUF"), ("unm", 1, "SBUF"),
            ("small", 4, "SBUF"), ("oc", 3, "SBUF"),
            ("psw", 2, "PSUM"), ("ptr", 2, "PSUM"), ("p512", 2, "PSUM"),
        ]:
            pools[nm] = ctx.enter_context(tc.tile_pool(name=nm, bufs=bufs, space=space))
        env = dict(nc=nc, tc=tc, pools=pools, P=P, out_ext=out_ext,
                   u_own=u_own, u_table=u_table, sched=sched, dt=dt)
        _emit_program(env)

    nc.compile()
    return nc


# --------------------------------------------------------------------- entry

def kernel(gate_type, edge_src, edge_dst, edge_w, emb, W1_0, W2_0, b2_0,
           W1_rest, W2_rest, b2_rest):
    global LAST_EXEC_NS
    key = hashlib.sha1(
        np.ascontiguousarray(np.asarray(edge_dst, dtype=np.int64)).tobytes()
        + np.ascontiguousarray(np.asarray(edge_src, dtype=np.int64)).tobytes()
    ).hexdigest()

    sched, per_core = _schedule_and_arrays(
        gate_type, edge_src, edge_dst, edge_w, emb, W1_0, W2_0, b2_0,
        W1_rest, W2_rest, b2_rest)

    if key in _cache and _cache[key][1]["NT"] == sched["NT"]:
        nc = _cache[key][0]
    else:
        nc = _build_nc(sched)
        _cache.clear()
        _cache[key] = (nc, sched)

    res = run_bass_kernel_spmd(nc, per_core, core_ids=list(range(NC)),
                               trace=PROFILE)
    LAST_EXEC_NS = res.exec_time_ns

    out = np.empty((N_NODES, H), np.float32)
    for c in range(NC):
        out[c * NPC:(c + 1) * NPC] = res.results[c]["out"][:, :NPC].T
    return out


# revision 18
# speedup vs baseline: 1.2891x; 1.2153x over previous
"""Trainium2 Bass kernel for nn_ActorCritic (3-layer edge-GNN, qconv stack).

Strategy (8 NeuronCores):
  - Nodes sharded 8 ways by dst: core c owns nodes [c*6250, (c+1)*6250).
  - Edges assigned to the core owning their dst; segment-sum is core-local.
  - Per layer l: u_l = h_{l-1} @ W1_l[:, :F].T is computed per-core on own
    nodes and AllGathered into a DRAM table; per-edge work is
      t_e = leaky_relu(u_l[src_e] + v_e),   v_e = w_e @ W1_l[:, F:].T  (host)
    with u_l[src_e] fetched by dma_gather (4 SWDGE queues).
  - Segment-sum by dst via per-tile selection-matrix matmuls into PSUM
    node-window accumulators (edges pre-sorted by (src<LO, dst-block)).
  - combine: h_l = relu(W2 @ [h; h_N] + b2) on TensorE, feature-major.
Host precomputes all index/layout arrays; the harness-visible entry point is
kernel(**inputs) -> np.ndarray [50000, 128] float32.
"""

import hashlib
import numpy as np
import ml_dtypes

import concourse.bass as bass
import concourse.bacc as bacc
import concourse.tile as tile
import concourse.mybir as mybir
from concourse.bass_utils import run_bass_kernel_spmd

BF16 = ml_dtypes.bfloat16
F32 = np.float32

N_NODES = 50000
N_EDGES = 800000
F0 = 32           # input feats (num gate types)
H = 128           # hidden dim
NC = 8            # cores
NPC = N_NODES // NC      # 6250 nodes per core
NB = 49                  # dst 128-blocks per core (49*128 = 6272)
NBW = NB * 128           # padded own-node count
SPLIT = 3072             # per-rank node split (block-aligned): lo = (src % NPC) < SPLIT
HI_R = NPC - SPLIT       # 3178 hi rows per rank
TILE = 128
CALL_TILES = 24          # tiles per dma_gather call (3072 rows)
PRELU_ALPHA = 0.01

PROFILE = False          # set True (e.g. from test.py) to capture HW timing
LAST_EXEC_NS = None

_cache = {}


# ----------------------------------------------------------------- host prep

def _schedule_and_arrays(gate_type, edge_src, edge_dst, edge_w,
                         emb, W1_0, W2_0, b2_0, W1_rest, W2_rest, b2_rest):
    src_all = np.asarray(edge_src).astype(np.int64)
    dst_all = np.asarray(edge_dst).astype(np.int64)
    gt_all = np.asarray(gate_type).astype(np.int64)
    w_all = np.asarray(edge_w).astype(np.float32)
    core_of = dst_all // NPC

    # per-core edge sets
    pc = []
    for c in range(NC):
        m = core_of == c
        s = src_all[m]
        dl = dst_all[m] - c * NPC
        pc.append((s, dl, w_all[m]))

    # counts per (core, pass, block);  pass 0 = src<LO, pass 1 = src>=LO
    cnt = np.zeros((NC, 2, NB), np.int64)
    for c in range(NC):
        s, dl, _ = pc[c]
        hi = ((s % NPC) >= SPLIT).astype(np.int64)
        key = hi * NB + dl // TILE
        bc = np.bincount(key, minlength=2 * NB)
        cnt[c] = bc.reshape(2, NB)

    ntiles = np.ceil(cnt.max(axis=0) / TILE).astype(np.int64)  # [2, NB]
    ntiles[0] = np.maximum(ntiles[0], 1)   # every block gets a pass-0 session

    # global tile list: pass 0 blocks 0..NB-1, then pass 1
    tiles = []          # (pass, block)
    sess_start = []
    sess_end = []
    tile_base = {}      # (p, b) -> first tile index
    for p in (0, 1):
        for b in range(NB):
            nt = int(ntiles[p][b])
            if nt == 0:
                continue
            tile_base[(p, b)] = len(tiles)
            for j in range(nt):
                tiles.append((p, b))
                sess_start.append(j == 0)
                sess_end.append(j == nt - 1)
    NT = len(tiles)
    pass0_tiles = int(ntiles[0].sum())

    # gather calls: chunks of CALL_TILES within each pass
    calls = []          # (pass, t0, t1)
    for p, lo_t, hi_t in ((0, 0, pass0_tiles), (1, pass0_tiles, NT)):
        t = lo_t
        while t < hi_t:
            t1 = min(t + CALL_TILES, hi_t)
            calls.append((p, t, t1))
            t = t1

    # per-core slot assignment + host arrays
    group_base = np.zeros(2 * NB, np.int64)
    for p in (0, 1):
        for b in range(NB):
            if (p, b) in tile_base:
                group_base[p * NB + b] = tile_base[(p, b)] * TILE

    W1w = [np.asarray(W1_0)[:, F0:F0 + 3],
           np.asarray(W1_rest)[0][:, H:H + 3],
           np.asarray(W1_rest)[1][:, H:H + 3]]
    h0_full = np.asarray(emb)[gt_all]          # [N, F0]

    per_core = []
    for c in range(NC):
        s, dl, w = pc[c]
        cs = s // NPC
        dls = s % NPC
        hi = (dls >= SPLIT).astype(np.int64)
        key = hi * NB + dl // TILE
        order = np.argsort(key, kind="stable")
        ks = key[order]
        first = np.zeros(2 * NB, np.int64)
        np.cumsum(np.bincount(ks, minlength=2 * NB)[:-1], out=first[1:])
        rank = np.arange(len(ks)) - first[ks]
        slot = group_base[ks] + rank           # global slot per sorted edge

        so, dlo, wo = s[order], dl[order], w[order]
        cso, dlso = cs[order], dls[order]
        idx12 = np.zeros(NT * TILE, np.int16)
        idx12[slot] = np.where(dlso < SPLIT, cso * SPLIT + dlso,
                               cso * HI_R + (dlso - SPLIT)).astype(np.int16)
        idx0 = np.zeros(NT * TILE, np.int16)
        idx0[slot] = gt_all[so].astype(np.int16)
        dstloc = np.full(NT * TILE, 200.0, np.float32)
        dstloc[slot] = (dlo % TILE).astype(np.float32)

        vs = []
        for l in range(3):
            vfull = np.zeros((NT * TILE, H), np.float32)
            vfull[slot] = wo @ W1w[l].T
            vs.append(np.ascontiguousarray(
                vfull.reshape(NT, TILE, H).transpose(1, 0, 2)).astype(BF16))

        def wrap(a):
            outs = []
            for (_, t0, t1) in calls:
                seg = a[t0 * TILE:t1 * TILE].reshape(-1, 16).T  # [16, ct*8]
                outs.append(np.tile(seg, (8, 1)))
            return np.ascontiguousarray(np.concatenate(outs, axis=1))

        cnts = np.bincount(dl, minlength=NBW).astype(np.float32)
        inv_cnt = (1.0 / np.maximum(cnts, 1.0)).reshape(NB, TILE).T  # [128, NB]

        h0T = np.zeros((F0, NBW), np.float32)
        h0T[:, :NPC] = h0_full[c * NPC:(c + 1) * NPC].T

        per_core.append({
            "idx12": wrap(idx12),
            "idx0": wrap(idx0),
            "dstloc": np.ascontiguousarray(
                dstloc.reshape(NT, TILE).T).astype(BF16),
            "v0": vs[0], "v1": vs[1], "v2": vs[2],
            "inv_cnt": np.ascontiguousarray(inv_cnt).astype(F32),
            "h0T": h0T.astype(BF16),
        })

    # shared weights
    table0 = (np.asarray(emb) @ np.asarray(W1_0)[:, :F0].T).astype(BF16)  # [32,128]
    w1ht = np.stack([np.asarray(W1_rest)[0][:, :H].T,
                     np.asarray(W1_rest)[1][:, :H].T]).astype(BF16)
    w2at0 = np.asarray(W2_0)[:, :F0].T.astype(BF16)        # [32, 128]
    w2bt0 = np.asarray(W2_0)[:, F0:].T.astype(BF16)        # [128, 128]
    w2at12 = np.stack([np.asarray(W2_rest)[0][:, :H].T,
                       np.asarray(W2_rest)[1][:, :H].T]).astype(BF16)
    w2bt12 = np.stack([np.asarray(W2_rest)[0][:, H:].T,
                       np.asarray(W2_rest)[1][:, H:].T]).astype(BF16)
    b2t = np.stack([np.asarray(b2_0),
                    np.asarray(b2_rest)[0],
                    np.asarray(b2_rest)[1]]).T.astype(F32)  # [128, 3]
    iota_row = np.tile(np.arange(TILE, dtype=np.float32), (TILE, 1)).astype(BF16)
    ident = np.eye(TILE, dtype=np.float32).astype(BF16)

    shared = {
        "table0": table0, "w1ht": w1ht,
        "w2at0": w2at0, "w2bt0": w2bt0,
        "w2at12": w2at12, "w2bt12": w2bt12,
        "b2t": b2t, "iota": iota_row, "ident": ident,
    }
    for m in per_core:
        m.update(shared)

    sched = {
        "NT": NT, "tiles": tiles, "sess_start": sess_start,
        "sess_end": sess_end, "calls": calls,
    }
    return sched, per_core


# ------------------------------------------------------------------- codegen

def _ap3(ap2, inner):
    """[P, K] AP -> [P, K, inner] AP broadcast along a new 0-stride inner."""
    return bass.AP(ap2.tensor, ap2.offset,
                   [list(ap2.ap[0]), list(ap2.ap[1]), [0, inner]])


def _emit_prologue(env):
    nc, pools, P, dt = env["nc"], env["pools"], env["P"], env["dt"]
    NT = env["sched"]["NT"]
    constp = pools["const"]

    def load_const(name, shape, dtyp):
        t_ = constp.tile(shape, dtyp, tag=name)
        nc.sync.dma_start(t_[:], P[name][:])
        return t_

    env["iota_sb"] = load_const("iota", [128, 128], dt.bfloat16)
    env["ident_sb"] = load_const("ident", [128, 128], dt.bfloat16)
    env["dstloc_sb"] = load_const("dstloc", [128, NT], dt.bfloat16)
    env["inv_sb"] = load_const("inv_cnt", [128, NB], dt.float32)
    env["b2_sb"] = load_const("b2t", [H, 3], dt.float32)
    env["w2at0_sb"] = load_const("w2at0", [F0, H], dt.bfloat16)
    env["w2bt0_sb"] = load_const("w2bt0", [H, H], dt.bfloat16)
    for nm in ("w1ht", "w2at12", "w2bt12"):
        t_ = constp.tile([H, 2, H], dt.bfloat16, tag=nm)
        nc.sync.dma_start(t_[:], P[nm].ap().rearrange("a k m -> k a m"))
        env[nm + "_sb"] = t_
    idx12_sb = pools["idxr"].tile([128, NT * 8], dt.int16)
    nc.sync.dma_start(idx12_sb[:], P["idx12"][:])
    env["idx12_sb"] = idx12_sb
    h0T_sb = constp.tile([F0, NBW], dt.bfloat16, tag="h0T")
    nc.sync.dma_start(h0T_sb[:], P["h0T"][:])
    env["h_prevT"] = h0T_sb
    env["s_acc"] = pools["acc"].tile([128, NB, H], dt.float32, name="s_acc")
    # idx column offsets per call (wrapped layout)
    call_cols = []
    off = 0
    for (_, t0, t1) in env["sched"]["calls"]:
        ct = t1 - t0
        call_cols.append(off)
        off += ct * 8
    env["call_cols"] = call_cols


def _emit_allgather(env, part):
    nc = env["nc"]
    if part == 0:
        ins = env["u_own_lo"][:, :].opt()
        outs = env["u_table_lo"][:, :].opt()
    else:
        ins = env["u_own_hi"][0:HI_R, :].opt()
        outs = env["u_table_hi"][:, :].opt()
    nc.gpsimd.collective_compute(
        "AllGather", mybir.AluOpType.bypass,
        replica_groups=[list(range(NC))],
        ins=[ins], outs=[outs],
    )


def _emit_edge_tile(env, l, gt, tt_ap, S_ap, state):
    """Selection matmul + session bookkeeping for one 128-edge tile."""
    nc, pools, dt = env["nc"], env["pools"], env["dt"]
    sched = env["sched"]
    pss_t, b = sched["tiles"][gt]
    if sched["sess_start"][gt]:
        state["psum"] = pools["psw"].tile([128, 128], dt.float32, name="psw")
    nc.tensor.matmul(state["psum"][:], S_ap, tt_ap,
                     start=bool(sched["sess_start"][gt]),
                     stop=bool(sched["sess_end"][gt]))
    if sched["sess_end"][gt]:
        s_acc = env["s_acc"]
        if pss_t == 0:
            nc.vector.tensor_copy(s_acc[:, b, :], state["psum"][:])
        else:
            nc.vector.tensor_tensor(
                out=s_acc[:, b, :], in0=s_acc[:, b, :],
                in1=state["psum"][:], op=mybir.AluOpType.add)


def _emit_call(env, l, ci, state):
    nc, pools, dt = env["nc"], env["pools"], env["dt"]
    pss, t0, t1 = env["sched"]["calls"][ci]
    ct = t1 - t0
    coff = env["call_cols"][ci]
    if l == 0:
        ixt = pools["idxs"].tile([128, CALL_TILES * 8], dt.int16, tag="ix")
        nc.sync.dma_start(ixt[:, :ct * 8], env["P"]["idx0"][:, coff:coff + ct * 8])
        idx_ap = ixt[:, :ct * 8]
        view = env["P"]["table0"][:]
    else:
        idx_ap = env["idx12_sb"][:, coff:coff + ct * 8]
        view = env["u_table_lo"][:] if pss == 0 else env["u_table_hi"][:]

    g = pools["g"].tile([128, CALL_TILES, H], dt.bfloat16, tag="g")
    nc.gpsimd.dma_gather(
        out_ap=g[:, :ct, :], in_ap=view, idxs_ap=idx_ap,
        num_idxs=ct * 128, num_idxs_reg=ct * 128, elem_size=H,
        single_packet=False, queue_num=ci % 4,
    )
    vsl = pools["vsl"].tile([128, CALL_TILES, H], dt.bfloat16, tag="v")
    nc.sync.dma_start(vsl[:, :ct, :], env["P"][f"v{l}"][:, t0:t1, :])

    import os
    if os.environ.get("GATHER_ONLY"):
        if ci == len(env["sched"]["calls"]) - 1:
            nc.vector.tensor_copy(env["s_acc"][:, 0, :], g[:, 0, :])
        return
    for j4 in range(0, ct, 4):
        nj = min(4, ct - j4)
        y = pools["y"].tile([128, 4, H], dt.bfloat16, tag="y")
        nc.vector.tensor_tensor(
            out=y[:, :nj, :], in0=g[:, j4:j4 + nj, :],
            in1=vsl[:, j4:j4 + nj, :], op=mybir.AluOpType.add)
        tt = pools["t"].tile([128, 4, H], dt.bfloat16, tag="t")
        nc.scalar.activation(tt[:, :nj, :], y[:, :nj, :],
                             mybir.ActivationFunctionType.Prelu,
                             alpha=PRELU_ALPHA)
        S4 = pools["S"].tile([128, 4, 128], dt.bfloat16, tag="S")
        gt0 = t0 + j4
        iap = env["iota_sb"][:, :]
        in0 = bass.AP(iap.tensor, iap.offset,
                      [list(iap.ap[0]), [0, nj], list(iap.ap[1])])
        dap = env["dstloc_sb"][:, gt0:gt0 + nj]
        in1 = bass.AP(dap.tensor, dap.offset,
                      [list(dap.ap[0]), list(dap.ap[1]), [0, 128]])
        nc.vector.tensor_tensor(out=S4[:, :nj, :], in0=in0, in1=in1,
                                op=mybir.AluOpType.is_equal)
        for jj in range(nj):
            _emit_edge_tile(env, l, gt0 + jj, tt[:, jj, :], S4[:, jj, :], state)


def _emit_chunk(env, l, k):
    """Epilogue for node chunk k (blocks 4k..): h_N scale+transpose, combine,
    and (l<2) next-layer u rows + store."""
    nc, pools, dt = env["nc"], env["pools"], env["dt"]
    b0 = 4 * k
    nb = min(4, NB - b0)
    ck0, ck = 512 * k, 128 * nb
    s_acc, inv = env["s_acc"], env["inv_sb"]

    hn = pools["small"].tile([128, 4, 128], dt.bfloat16, tag="hn")
    iap = inv[:, b0:b0 + nb]
    in1 = bass.AP(iap.tensor, iap.offset,
                  [list(iap.ap[0]), list(iap.ap[1]), [0, 128]])
    nc.vector.tensor_tensor(out=hn[:, :nb, :], in0=s_acc[:, b0:b0 + nb, :],
                            in1=in1, op=mybir.AluOpType.mult)
    hNT = pools["scr"].tile([H, 512], dt.bfloat16, tag="hNT")
    for j in range(nb):
        ptr = pools["ptr"].tile([128, 128], dt.bfloat16, tag="ptr", name="ptr")
        nc.tensor.transpose(ptr[:], hn[:, j, :], env["ident_sb"][:])
        nc.vector.tensor_copy(hNT[:, j * 128:(j + 1) * 128], ptr[:])

    if l == 0:
        w2a, w2b = env["w2at0_sb"][:], env["w2bt0_sb"][:]
    else:
        w2a = env["w2at12_sb"][:, l - 1, :]
        w2b = env["w2bt12_sb"][:, l - 1, :]
    bias = env["b2_sb"][:, l:l + 1]
    pc_ = pools["p512"].tile([128, 512], dt.float32, tag="p512", name="pc_")
    nc.tensor.matmul(pc_[:, :ck], w2a, env["h_prevT"][:, ck0:ck0 + ck],
                     start=True, stop=False)
    nc.tensor.matmul(pc_[:, :ck], w2b, hNT[:, :ck], start=False, stop=True)
    if l < 2:
        h_outT = env["h_outT"]
        nc.scalar.activation(h_outT[:, ck0:ck0 + ck], pc_[:, :ck],
                             mybir.ActivationFunctionType.Relu, bias=bias)
        # next-layer u rows for this chunk: u = W1h_{l+1} @ h_out
        pu = pools["p512"].tile([128, 512], dt.float32, tag="p512", name="pu")
        nc.tensor.matmul(pu[:, :ck], env["w1ht_sb"][:, l, :],
                         h_outT[:, ck0:ck0 + ck], start=True, stop=True)
        uTc = pools["scr"].tile([H, 512], dt.bfloat16, tag="uTc")
        nc.scalar.activation(uTc[:, :ck], pu[:, :ck],
                             mybir.ActivationFunctionType.Copy)
        u_nm = pools["unm"].tile([128, 4, H], dt.bfloat16, tag="unm")
        for j in range(nb):
            ptru = pools["ptr"].tile([128, 128], dt.bfloat16, tag="ptr", name="ptru")
            nc.tensor.transpose(ptru[:], uTc[:, j * 128:(j + 1) * 128],
                                env["ident_sb"][:])
            nc.vector.tensor_copy(u_nm[:, j, :], ptru[:])
            b = b0 + j
            if b < SPLIT // 128:
                dst = env["u_own_lo"][b * 128:(b + 1) * 128, :]
            else:
                bb = b - SPLIT // 128
                dst = env["u_own_hi"][bb * 128:(bb + 1) * 128, :]
            nc.sync.dma_start(dst, u_nm[:, j, :])
    else:
        oc = pools["oc"].tile([H, 512], dt.float32, tag="oc")
        nc.scalar.activation(oc[:, :ck], pc_[:, :ck],
                             mybir.ActivationFunctionType.Relu, bias=bias)
        nc.sync.dma_start(env["out_ext"][:, ck0:ck0 + ck], oc[:, :ck])


def _emit_program(env):
    _emit_prologue(env)
    sched = env["sched"]
    ncalls = len(sched["calls"])
    NCH = (NB + 3) // 4

    # per-block final tile = end of its last session; chunk ready-tile
    final_tile = {}
    for gt, (p, b) in enumerate(sched["tiles"]):
        if sched["sess_end"][gt]:
            final_tile[b] = gt
    chunk_ready = [max(final_tile[b] for b in range(4 * k, min(4 * k + 4, NB)))
                   for k in range(NCH)]

    pools, dt = env["pools"], env["dt"]
    for l in range(3):
        if l < 2:
            env["h_outT"] = pools["h"].tile([H, NBW], dt.bfloat16, tag="h",
                                            name=f"h{l + 1}")
        nxt = 0
        state = {}

        def emit_chunks_until(bound):
            nonlocal nxt
            while nxt < bound:
                _emit_chunk(env, l, nxt)
                nxt += 1
                if l < 2 and nxt == SPLIT // 512:
                    _emit_allgather(env, 0)

        for ci in range(ncalls):
            _emit_call(env, l, ci, state)
            t1 = sched["calls"][ci][2]
            while nxt < NCH and chunk_ready[nxt] < t1:
                emit_chunks_until(nxt + 1)
        emit_chunks_until(NCH)
        if l < 2:
            _emit_allgather(env, 1)
            env["h_prevT"] = env["h_outT"]


def _build_nc(sched):
    NT = sched["NT"]
    dt = mybir.dt

    nc = bacc.Bacc("TRN2", target_bir_lowering=False, debug=False,
                   num_devices=NC, num_swdge_queues=4)

    P = {}
    P["idx12"] = nc.dram_tensor("idx12", [128, NT * 8], dt.int16, kind="ExternalInput")
    P["idx0"] = nc.dram_tensor("idx0", [128, NT * 8], dt.int16, kind="ExternalInput")
    P["dstloc"] = nc.dram_tensor("dstloc", [128, NT], dt.bfloat16, kind="ExternalInput")
    for l in range(3):
        P[f"v{l}"] = nc.dram_tensor(f"v{l}", [128, NT, H], dt.bfloat16, kind="ExternalInput")
    P["inv_cnt"] = nc.dram_tensor("inv_cnt", [128, NB], dt.float32, kind="ExternalInput")
    P["h0T"] = nc.dram_tensor("h0T", [F0, NBW], dt.bfloat16, kind="ExternalInput")
    P["table0"] = nc.dram_tensor("table0", [F0, H], dt.bfloat16, kind="ExternalInput")
    P["w1ht"] = nc.dram_tensor("w1ht", [2, H, H], dt.bfloat16, kind="ExternalInput")
    P["w2at0"] = nc.dram_tensor("w2at0", [F0, H], dt.bfloat16, kind="ExternalInput")
    P["w2bt0"] = nc.dram_tensor("w2bt0", [H, H], dt.bfloat16, kind="ExternalInput")
    P["w2at12"] = nc.dram_tensor("w2at12", [2, H, H], dt.bfloat16, kind="ExternalInput")
    P["w2bt12"] = nc.dram_tensor("w2bt12", [2, H, H], dt.bfloat16, kind="ExternalInput")
    P["b2t"] = nc.dram_tensor("b2t", [H, 3], dt.float32, kind="ExternalInput")
    P["iota"] = nc.dram_tensor("iota", [128, 128], dt.bfloat16, kind="ExternalInput")
    P["ident"] = nc.dram_tensor("ident", [128, 128], dt.bfloat16, kind="ExternalInput")

    out_ext = nc.dram_tensor("out", [H, NBW], dt.float32, kind="ExternalOutput")
    u_own_lo = nc.dram_tensor("u_own_lo", [SPLIT, H], dt.bfloat16)
    u_own_hi = nc.dram_tensor("u_own_hi", [3200, H], dt.bfloat16)
    u_table_lo = nc.dram_tensor("u_table_lo", [NC * SPLIT, H], dt.bfloat16, addr_space="Shared")
    u_table_hi = nc.dram_tensor("u_table_hi", [NC * HI_R, H], dt.bfloat16, addr_space="Shared")

    from contextlib import ExitStack
    with tile.TileContext(nc) as tc, ExitStack() as ctx:
        pools = {}
        for nm, bufs, space in [
            ("const", 1, "SBUF"), ("idxr", 1, "SBUF"), ("idxs", 4, "SBUF"),
            ("g", 8, "SBUF"), ("vsl", 4, "SBUF"), ("y", 6, "SBUF"),
            ("t", 6, "SBUF"), ("S", 8, "SBUF"), ("acc", 1, "SBUF"),
            ("h", 2, "SBUF"), ("scr", 2, "SBUF"), ("unm", 1, "SBUF"),
            ("small", 4, "SBUF"), ("oc", 3, "SBUF"),
            ("psw", 2, "PSUM"), ("ptr", 2, "PSUM"), ("p512", 2, "PSUM"),
        ]:
            pools[nm] = ctx.enter_context(tc.tile_pool(name=nm, bufs=bufs, space=space))
        env = dict(nc=nc, tc=tc, pools=pools, P=P, out_ext=out_ext,
                   u_own_lo=u_own_lo, u_own_hi=u_own_hi,
                   u_table_lo=u_table_lo, u_table_hi=u_table_hi,
                   sched=sched, dt=dt)
        _emit_program(env)

    nc.compile()
    return nc


# --------------------------------------------------------------------- entry

def kernel(gate_type, edge_src, edge_dst, edge_w, emb, W1_0, W2_0, b2_0,
           W1_rest, W2_rest, b2_rest):
    global LAST_EXEC_NS
    key = hashlib.sha1(
        np.ascontiguousarray(np.asarray(edge_dst, dtype=np.int64)).tobytes()
        + np.ascontiguousarray(np.asarray(edge_src, dtype=np.int64)).tobytes()
    ).hexdigest()

    sched, per_core = _schedule_and_arrays(
        gate_type, edge_src, edge_dst, edge_w, emb, W1_0, W2_0, b2_0,
        W1_rest, W2_rest, b2_rest)

    if key in _cache and _cache[key][1]["NT"] == sched["NT"]:
        nc = _cache[key][0]
    else:
        nc = _build_nc(sched)
        _cache.clear()
        _cache[key] = (nc, sched)

    res = run_bass_kernel_spmd(nc, per_core, core_ids=list(range(NC)),
                               trace=PROFILE)
    LAST_EXEC_NS = res.exec_time_ns

    out = np.empty((N_NODES, H), np.float32)
    for c in range(NC):
        out[c * NPC:(c + 1) * NPC] = res.results[c]["out"][:, :NPC].T
    return out


# revision 22
# speedup vs baseline: 1.4069x; 1.0913x over previous
"""Trainium2 Bass kernel for nn_ActorCritic (3-layer edge-GNN, qconv stack).

Strategy (8 NeuronCores):
  - Nodes sharded 8 ways by dst: core c owns nodes [c*6250, (c+1)*6250).
  - Edges assigned to the core owning their dst; segment-sum is core-local.
  - Per layer l: u_l = h_{l-1} @ W1_l[:, :F].T is computed per-core on own
    nodes and AllGathered into a DRAM table; per-edge work is
      t_e = leaky_relu(u_l[src_e] + v_e),   v_e = w_e @ W1_l[:, F:].T  (host)
    with u_l[src_e] fetched by dma_gather (4 SWDGE queues).
  - Segment-sum by dst via per-tile selection-matrix matmuls into PSUM
    node-window accumulators (edges pre-sorted by (src<LO, dst-block)).
  - combine: h_l = relu(W2 @ [h; h_N] + b2) on TensorE, feature-major.
Host precomputes all index/layout arrays; the harness-visible entry point is
kernel(**inputs) -> np.ndarray [50000, 128] float32.
"""

import hashlib
import numpy as np
import ml_dtypes

import concourse.bass as bass
import concourse.bacc as bacc
import concourse.tile as tile
import concourse.mybir as mybir
from concourse.bass_utils import run_bass_kernel_spmd

BF16 = ml_dtypes.bfloat16
F32 = np.float32

N_NODES = 50000
N_EDGES = 800000
F0 = 32           # input feats (num gate types)
H = 128           # hidden dim
NC = 8            # cores
NPC = N_NODES // NC      # 6250 nodes per core
NB = 49                  # dst 128-blocks per core (49*128 = 6272)
NBW = NB * 128           # padded own-node count
SPLIT = 3072             # per-rank node split (block-aligned): lo = (src % NPC) < SPLIT
HI_R = NPC - SPLIT       # 3178 hi rows per rank
TILE = 128
CALL_TILES = 24          # tiles per dma_gather call (3072 rows)
PRELU_ALPHA = 0.01

PROFILE = False          # set True (e.g. from test.py) to capture HW timing
LAST_EXEC_NS = None

_cache = {}


# ----------------------------------------------------------------- host prep

def _schedule_and_arrays(gate_type, edge_src, edge_dst, edge_w,
                         emb, W1_0, W2_0, b2_0, W1_rest, W2_rest, b2_rest):
    src_all = np.asarray(edge_src).astype(np.int64)
    dst_all = np.asarray(edge_dst).astype(np.int64)
    gt_all = np.asarray(gate_type).astype(np.int64)
    w_all = np.asarray(edge_w).astype(np.float32)
    core_of = dst_all // NPC

    # per-core edge sets
    pc = []
    for c in range(NC):
        m = core_of == c
        s = src_all[m]
        dl = dst_all[m] - c * NPC
        pc.append((s, dl, w_all[m]))

    # counts per (core, pass, block);  pass 0 = src<LO, pass 1 = src>=LO
    cnt = np.zeros((NC, 2, NB), np.int64)
    for c in range(NC):
        s, dl, _ = pc[c]
        hi = ((s % NPC) >= SPLIT).astype(np.int64)
        key = hi * NB + dl // TILE
        bc = np.bincount(key, minlength=2 * NB)
        cnt[c] = bc.reshape(2, NB)

    ntiles = np.ceil(cnt.max(axis=0) / TILE).astype(np.int64)  # [2, NB]
    ntiles[0] = np.maximum(ntiles[0], 1)   # every block gets a pass-0 session

    # global tile list: pass 0 blocks 0..NB-1, then pass 1
    tiles = []          # (pass, block)
    sess_start = []
    sess_end = []
    tile_base = {}      # (p, b) -> first tile index
    for p in (0, 1):
        for b in range(NB):
            nt = int(ntiles[p][b])
            if nt == 0:
                continue
            tile_base[(p, b)] = len(tiles)
            for j in range(nt):
                tiles.append((p, b))
                sess_start.append(j == 0)
                sess_end.append(j == nt - 1)
    NT = len(tiles)
    pass0_tiles = int(ntiles[0].sum())

    # gather calls: chunks of CALL_TILES within each pass
    calls = []          # (pass, t0, t1)
    for p, lo_t, hi_t in ((0, 0, pass0_tiles), (1, pass0_tiles, NT)):
        t = lo_t
        while t < hi_t:
            t1 = min(t + CALL_TILES, hi_t)
            calls.append((p, t, t1))
            t = t1

    # per-core slot assignment + host arrays
    group_base = np.zeros(2 * NB, np.int64)
    for p in (0, 1):
        for b in range(NB):
            if (p, b) in tile_base:
                group_base[p * NB + b] = tile_base[(p, b)] * TILE

    W1w = [np.asarray(W1_0)[:, F0:F0 + 3],
           np.asarray(W1_rest)[0][:, H:H + 3],
           np.asarray(W1_rest)[1][:, H:H + 3]]
    h0_full = np.asarray(emb)[gt_all]          # [N, F0]

    per_core = []
    for c in range(NC):
        s, dl, w = pc[c]
        cs = s // NPC
        dls = s % NPC
        hi = (dls >= SPLIT).astype(np.int64)
        key = hi * NB + dl // TILE
        order = np.argsort(key, kind="stable")
        ks = key[order]
        first = np.zeros(2 * NB, np.int64)
        np.cumsum(np.bincount(ks, minlength=2 * NB)[:-1], out=first[1:])
        rank = np.arange(len(ks)) - first[ks]
        slot = group_base[ks] + rank           # global slot per sorted edge

        so, dlo, wo = s[order], dl[order], w[order]
        cso, dlso = cs[order], dls[order]
        idx12 = np.zeros(NT * TILE, np.int16)
        idx12[slot] = np.where(dlso < SPLIT, cso * SPLIT + dlso,
                               cso * HI_R + (dlso - SPLIT)).astype(np.int16)
        gt_dense = np.full(NT * TILE, 255.0, np.float32)
        gt_dense[slot] = gt_all[so].astype(np.float32)
        gt_rep = np.ascontiguousarray(np.broadcast_to(
            gt_dense.reshape(1, NT, TILE), (F0, NT, TILE))).astype(BF16)
        wfull = np.zeros((NT * TILE, 3), np.float32)
        wfull[slot] = wo
        wT_res = np.ascontiguousarray(
            wfull.reshape(NT, TILE, 3).transpose(2, 0, 1)).astype(BF16)
        dstloc = np.full(NT * TILE, 200.0, np.float32)
        dstloc[slot] = (dlo % TILE).astype(np.float32)

        vs = []
        for l in (1, 2):
            vfull2 = np.zeros((NT * TILE, H), np.float32)
            vfull2[slot] = wo @ W1w[l].T
            vs.append(np.ascontiguousarray(
                vfull2.reshape(NT, TILE, H).transpose(1, 0, 2)).astype(BF16))

        def wrap(a):
            outs = []
            for (_, t0, t1) in calls:
                seg = a[t0 * TILE:t1 * TILE].reshape(-1, 16).T  # [16, ct*8]
                outs.append(np.tile(seg, (8, 1)))
            return np.ascontiguousarray(np.concatenate(outs, axis=1))

        cnts = np.bincount(dl, minlength=NBW).astype(np.float32)
        inv_cnt = (1.0 / np.maximum(cnts, 1.0)).reshape(NB, TILE).T  # [128, NB]

        h0T = np.zeros((F0, NBW), np.float32)
        h0T[:, :NPC] = h0_full[c * NPC:(c + 1) * NPC].T

        per_core.append({
            "idx12": wrap(idx12),
            "gt_rep": gt_rep,
            "wt_res": wT_res,
            "dstloc": np.ascontiguousarray(
                dstloc.reshape(NT, TILE).T).astype(BF16),
            "v1": vs[0], "v2": vs[1],
            "inv_cnt": np.ascontiguousarray(inv_cnt).astype(F32),
            "h0T": h0T.astype(BF16),
        })

    # shared weights
    table0 = np.asarray(emb) @ np.asarray(W1_0)[:, :F0].T   # [32,128]
    rhs0 = np.concatenate([table0, np.asarray(W1_0)[:, F0:F0 + 3].T], axis=0).astype(BF16)  # [35,128]
    iota32 = np.arange(F0, dtype=np.float32)[:, None].astype(BF16)  # [32,1]
    w1ht = np.stack([np.asarray(W1_rest)[0][:, :H].T,
                     np.asarray(W1_rest)[1][:, :H].T]).astype(BF16)
    w2at0 = np.asarray(W2_0)[:, :F0].T.astype(BF16)        # [32, 128]
    w2bt0 = np.asarray(W2_0)[:, F0:].T.astype(BF16)        # [128, 128]
    w2at12 = np.stack([np.asarray(W2_rest)[0][:, :H].T,
                       np.asarray(W2_rest)[1][:, :H].T]).astype(BF16)
    w2bt12 = np.stack([np.asarray(W2_rest)[0][:, H:].T,
                       np.asarray(W2_rest)[1][:, H:].T]).astype(BF16)
    b2t = np.stack([np.asarray(b2_0),
                    np.asarray(b2_rest)[0],
                    np.asarray(b2_rest)[1]]).T.astype(F32)  # [128, 3]
    iota_row = np.tile(np.arange(TILE, dtype=np.float32), (TILE, 1)).astype(BF16)
    ident = np.eye(TILE, dtype=np.float32).astype(BF16)

    shared = {
        "rhs0": rhs0, "iota32": iota32, "w1ht": w1ht,
        "w2at0": w2at0, "w2bt0": w2bt0,
        "w2at12": w2at12, "w2bt12": w2bt12,
        "b2t": b2t, "iota": iota_row, "ident": ident,
    }
    for m in per_core:
        m.update(shared)

    sched = {
        "NT": NT, "tiles": tiles, "sess_start": sess_start,
        "sess_end": sess_end, "calls": calls,
    }
    return sched, per_core


# ------------------------------------------------------------------- codegen

def _ap3(ap2, inner):
    """[P, K] AP -> [P, K, inner] AP broadcast along a new 0-stride inner."""
    return bass.AP(ap2.tensor, ap2.offset,
                   [list(ap2.ap[0]), list(ap2.ap[1]), [0, inner]])


def _emit_prologue(env):
    nc, pools, P, dt = env["nc"], env["pools"], env["P"], env["dt"]
    NT = env["sched"]["NT"]
    constp = pools["const"]

    def load_const(name, shape, dtyp):
        t_ = constp.tile(shape, dtyp, tag=name)
        nc.sync.dma_start(t_[:], P[name][:])
        return t_

    env["iota_sb"] = load_const("iota", [128, 128], dt.bfloat16)
    env["ident_sb"] = load_const("ident", [128, 128], dt.bfloat16)
    env["dstloc_sb"] = load_const("dstloc", [128, NT], dt.bfloat16)
    env["inv_sb"] = load_const("inv_cnt", [128, NB], dt.float32)
    env["b2_sb"] = load_const("b2t", [H, 3], dt.float32)
    env["rhs0_sb"] = load_const("rhs0", [F0 + 3, H], dt.bfloat16)
    env["iota32_sb"] = load_const("iota32", [F0, 1], dt.bfloat16)
    env["w2at0_sb"] = load_const("w2at0", [F0, H], dt.bfloat16)
    env["w2bt0_sb"] = load_const("w2bt0", [H, H], dt.bfloat16)
    for nm in ("w1ht", "w2at12", "w2bt12"):
        t_ = constp.tile([H, 2, H], dt.bfloat16, tag=nm)
        nc.sync.dma_start(t_[:], P[nm].ap().rearrange("a k m -> k a m"))
        env[nm + "_sb"] = t_
    idx12_sb = pools["idxr"].tile([128, NT * 8], dt.int16)
    nc.sync.dma_start(idx12_sb[:], P["idx12"][:])
    env["idx12_sb"] = idx12_sb
    h0T_sb = constp.tile([F0, NBW], dt.bfloat16, tag="h0T")
    nc.sync.dma_start(h0T_sb[:], P["h0T"][:])
    env["h_prevT"] = h0T_sb
    env["s_acc"] = pools["acc"].tile([128, NB, H], dt.float32, name="s_acc")
    # idx column offsets per call (wrapped layout)
    call_cols = []
    off = 0
    for (_, t0, t1) in env["sched"]["calls"]:
        ct = t1 - t0
        call_cols.append(off)
        off += ct * 8
    env["call_cols"] = call_cols


def _emit_allgather(env, part):
    nc = env["nc"]
    if part == 0:
        ins = env["u_own_lo"][:, :].opt()
        outs = env["u_table_lo"][:, :].opt()
    else:
        ins = env["u_own_hi"][0:HI_R, :].opt()
        outs = env["u_table_hi"][:, :].opt()
    nc.gpsimd.collective_compute(
        "AllGather", mybir.AluOpType.bypass,
        replica_groups=[list(range(NC))],
        ins=[ins], outs=[outs],
    )


def _emit_edge_tile(env, l, gt, tt_ap, S_ap, state):
    """Selection matmul + session bookkeeping for one 128-edge tile."""
    nc, pools, dt = env["nc"], env["pools"], env["dt"]
    sched = env["sched"]
    pss_t, b = sched["tiles"][gt]
    if sched["sess_start"][gt]:
        state["psum"] = pools["psw"].tile([128, 128], dt.float32, name="psw")
    nc.tensor.matmul(state["psum"][:], S_ap, tt_ap,
                     start=bool(sched["sess_start"][gt]),
                     stop=bool(sched["sess_end"][gt]))
    if sched["sess_end"][gt]:
        s_acc = env["s_acc"]
        if pss_t == 0:
            nc.vector.tensor_copy(s_acc[:, b, :], state["psum"][:])
        else:
            nc.vector.tensor_tensor(
                out=s_acc[:, b, :], in0=s_acc[:, b, :],
                in1=state["psum"][:], op=mybir.AluOpType.add)


def _emit_call(env, l, ci, state):
    nc, pools, dt = env["nc"], env["pools"], env["dt"]
    pss, t0, t1 = env["sched"]["calls"][ci]
    ct = t1 - t0
    coff = env["call_cols"][ci]
    if l == 0:
        gts = pools["g"].tile([F0, CALL_TILES, TILE], dt.bfloat16, tag="gts", bufs=3)
        nc.sync.dma_start(gts[:, :ct, :], env["P"]["gt_rep"][:, t0:t1, :])
        s0c = pools["S0"].tile([F0 + 3, CALL_TILES, TILE], dt.bfloat16, tag="S0", bufs=3)
        nc.sync.dma_start(s0c[F0:F0 + 3, :ct, :], env["P"]["wt_res"][:, t0:t1, :])
    else:
        idx_ap = env["idx12_sb"][:, coff:coff + ct * 8]
        view = env["u_table_lo"][:] if pss == 0 else env["u_table_hi"][:]
        g = pools["g"].tile([128, CALL_TILES, H], dt.bfloat16, tag="g")
        nc.gpsimd.dma_gather(
            out_ap=g[:, :ct, :], in_ap=view, idxs_ap=idx_ap,
            num_idxs=ct * 128, num_idxs_reg=ct * 128, elem_size=H,
            single_packet=False, queue_num=ci % 4,
        )
        vsl = pools["vsl"].tile([128, CALL_TILES, H], dt.bfloat16, tag="v")
        nc.sync.dma_start(vsl[:, :ct, :], env["P"][f"v{l}"][:, t0:t1, :])

    for j4 in range(0, ct, 4):
        nj = min(4, ct - j4)
        gt0 = t0 + j4
        tt = pools["t"].tile([128, 4, H], dt.bfloat16, tag="t")
        if l == 0:
            i32 = env["iota32_sb"][:, :]
            in1 = bass.AP(i32.tensor, i32.offset,
                          [list(i32.ap[0]), [0, nj], [0, TILE]])
            nc.vector.tensor_tensor(
                out=s0c[0:F0, j4:j4 + nj, :], in0=gts[:, j4:j4 + nj, :],
                in1=in1, op=mybir.AluOpType.is_equal)
            for jj in range(nj):
                pt0 = pools["pt0"].tile([128, TILE], dt.float32, tag="pt0",
                                        name="pt0")
                nc.tensor.matmul(pt0[:], s0c[:, j4 + jj, :], env["rhs0_sb"][:],
                                 start=True, stop=True)
                nc.scalar.activation(tt[:, jj, :], pt0[:],
                                     mybir.ActivationFunctionType.Prelu,
                                     alpha=PRELU_ALPHA)
        else:
            y = pools["y"].tile([128, 4, H], dt.bfloat16, tag="y")
            nc.vector.tensor_tensor(
                out=y[:, :nj, :], in0=g[:, j4:j4 + nj, :],
                in1=vsl[:, j4:j4 + nj, :], op=mybir.AluOpType.add)
            nc.scalar.activation(tt[:, :nj, :], y[:, :nj, :],
                                 mybir.ActivationFunctionType.Prelu,
                                 alpha=PRELU_ALPHA)
        S4 = pools["S"].tile([128, 4, 128], dt.bfloat16, tag="S")
        iap = env["iota_sb"][:, :]
        in0 = bass.AP(iap.tensor, iap.offset,
                      [list(iap.ap[0]), [0, nj], list(iap.ap[1])])
        dap = env["dstloc_sb"][:, gt0:gt0 + nj]
        in1 = bass.AP(dap.tensor, dap.offset,
                      [list(dap.ap[0]), list(dap.ap[1]), [0, 128]])
        nc.vector.tensor_tensor(out=S4[:, :nj, :], in0=in0, in1=in1,
                                op=mybir.AluOpType.is_equal)
        for jj in range(nj):
            _emit_edge_tile(env, l, gt0 + jj, tt[:, jj, :], S4[:, jj, :], state)


def _emit_chunk(env, l, k):
    """Epilogue for node chunk k (blocks 4k..): h_N scale+transpose, combine,
    and (l<2) next-layer u rows + store."""
    nc, pools, dt = env["nc"], env["pools"], env["dt"]
    b0 = 4 * k
    nb = min(4, NB - b0)
    ck0, ck = 512 * k, 128 * nb
    s_acc, inv = env["s_acc"], env["inv_sb"]

    hn = pools["small"].tile([128, 4, 128], dt.bfloat16, tag="hn")
    iap = inv[:, b0:b0 + nb]
    in1 = bass.AP(iap.tensor, iap.offset,
                  [list(iap.ap[0]), list(iap.ap[1]), [0, 128]])
    nc.vector.tensor_tensor(out=hn[:, :nb, :], in0=s_acc[:, b0:b0 + nb, :],
                            in1=in1, op=mybir.AluOpType.mult)
    hNT = pools["scr"].tile([H, 512], dt.bfloat16, tag="hNT")
    for j in range(nb):
        ptr = pools["ptr"].tile([128, 128], dt.bfloat16, tag="ptr", name="ptr")
        nc.tensor.transpose(ptr[:], hn[:, j, :], env["ident_sb"][:])
        nc.vector.tensor_copy(hNT[:, j * 128:(j + 1) * 128], ptr[:])

    if l == 0:
        w2a, w2b = env["w2at0_sb"][:], env["w2bt0_sb"][:]
    else:
        w2a = env["w2at12_sb"][:, l - 1, :]
        w2b = env["w2bt12_sb"][:, l - 1, :]
    bias = env["b2_sb"][:, l:l + 1]
    pc_ = pools["p512"].tile([128, 512], dt.float32, tag="p512", name="pc_")
    nc.tensor.matmul(pc_[:, :ck], w2a, env["h_prevT"][:, ck0:ck0 + ck],
                     start=True, stop=False)
    nc.tensor.matmul(pc_[:, :ck], w2b, hNT[:, :ck], start=False, stop=True)
    if l < 2:
        h_outT = env["h_outT"]
        nc.scalar.activation(h_outT[:, ck0:ck0 + ck], pc_[:, :ck],
                             mybir.ActivationFunctionType.Relu, bias=bias)
        # next-layer u rows for this chunk: u = W1h_{l+1} @ h_out
        pu = pools["p512"].tile([128, 512], dt.float32, tag="p512", name="pu")
        nc.tensor.matmul(pu[:, :ck], env["w1ht_sb"][:, l, :],
                         h_outT[:, ck0:ck0 + ck], start=True, stop=True)
        uTc = pools["scr"].tile([H, 512], dt.bfloat16, tag="uTc")
        nc.scalar.activation(uTc[:, :ck], pu[:, :ck],
                             mybir.ActivationFunctionType.Copy)
        u_nm = pools["unm"].tile([128, 4, H], dt.bfloat16, tag="unm")
        for j in range(nb):
            ptru = pools["ptr"].tile([128, 128], dt.bfloat16, tag="ptr", name="ptru")
            nc.tensor.transpose(ptru[:], uTc[:, j * 128:(j + 1) * 128],
                                env["ident_sb"][:])
            nc.vector.tensor_copy(u_nm[:, j, :], ptru[:])
            b = b0 + j
            if b < SPLIT // 128:
                dst = env["u_own_lo"][b * 128:(b + 1) * 128, :]
            else:
                bb = b - SPLIT // 128
                dst = env["u_own_hi"][bb * 128:(bb + 1) * 128, :]
            nc.sync.dma_start(dst, u_nm[:, j, :])
    else:
        oc = pools["oc"].tile([H, 512], dt.float32, tag="oc")
        nc.scalar.activation(oc[:, :ck], pc_[:, :ck],
                             mybir.ActivationFunctionType.Relu, bias=bias)
        nc.sync.dma_start(env["out_ext"][:, ck0:ck0 + ck], oc[:, :ck])


def _emit_program(env):
    _emit_prologue(env)
    sched = env["sched"]
    ncalls = len(sched["calls"])
    NCH = (NB + 3) // 4

    # per-block final tile = end of its last session; chunk ready-tile
    final_tile = {}
    for gt, (p, b) in enumerate(sched["tiles"]):
        if sched["sess_end"][gt]:
            final_tile[b] = gt
    chunk_ready = [max(final_tile[b] for b in range(4 * k, min(4 * k + 4, NB)))
                   for k in range(NCH)]

    pools, dt = env["pools"], env["dt"]
    for l in range(3):
        if l < 2:
            env["h_outT"] = pools["h"].tile([H, NBW], dt.bfloat16, tag="h",
                                            name=f"h{l + 1}")
        nxt = 0
        state = {}

        def emit_chunks_until(bound):
            nonlocal nxt
            while nxt < bound:
                _emit_chunk(env, l, nxt)
                nxt += 1
                if l < 2 and nxt == SPLIT // 512:
                    _emit_allgather(env, 0)

        for ci in range(ncalls):
            _emit_call(env, l, ci, state)
            t1 = sched["calls"][ci][2]
            while nxt < NCH and chunk_ready[nxt] < t1:
                emit_chunks_until(nxt + 1)
        emit_chunks_until(NCH)
        if l < 2:
            _emit_allgather(env, 1)
            env["h_prevT"] = env["h_outT"]


def _build_nc(sched):
    NT = sched["NT"]
    dt = mybir.dt

    nc = bacc.Bacc("TRN2", target_bir_lowering=False, debug=False,
                   num_devices=NC, num_swdge_queues=4)

    P = {}
    P["idx12"] = nc.dram_tensor("idx12", [128, NT * 8], dt.int16, kind="ExternalInput")
    P["gt_rep"] = nc.dram_tensor("gt_rep", [F0, NT, TILE], dt.bfloat16, kind="ExternalInput")
    P["wt_res"] = nc.dram_tensor("wt_res", [3, NT, TILE], dt.bfloat16, kind="ExternalInput")
    P["dstloc"] = nc.dram_tensor("dstloc", [128, NT], dt.bfloat16, kind="ExternalInput")
    for l in (1, 2):
        P[f"v{l}"] = nc.dram_tensor(f"v{l}", [128, NT, H], dt.bfloat16, kind="ExternalInput")
    P["inv_cnt"] = nc.dram_tensor("inv_cnt", [128, NB], dt.float32, kind="ExternalInput")
    P["h0T"] = nc.dram_tensor("h0T", [F0, NBW], dt.bfloat16, kind="ExternalInput")
    P["rhs0"] = nc.dram_tensor("rhs0", [F0 + 3, H], dt.bfloat16, kind="ExternalInput")
    P["iota32"] = nc.dram_tensor("iota32", [F0, 1], dt.bfloat16, kind="ExternalInput")
    P["w1ht"] = nc.dram_tensor("w1ht", [2, H, H], dt.bfloat16, kind="ExternalInput")
    P["w2at0"] = nc.dram_tensor("w2at0", [F0, H], dt.bfloat16, kind="ExternalInput")
    P["w2bt0"] = nc.dram_tensor("w2bt0", [H, H], dt.bfloat16, kind="ExternalInput")
    P["w2at12"] = nc.dram_tensor("w2at12", [2, H, H], dt.bfloat16, kind="ExternalInput")
    P["w2bt12"] = nc.dram_tensor("w2bt12", [2, H, H], dt.bfloat16, kind="ExternalInput")
    P["b2t"] = nc.dram_tensor("b2t", [H, 3], dt.float32, kind="ExternalInput")
    P["iota"] = nc.dram_tensor("iota", [128, 128], dt.bfloat16, kind="ExternalInput")
    P["ident"] = nc.dram_tensor("ident", [128, 128], dt.bfloat16, kind="ExternalInput")

    out_ext = nc.dram_tensor("out", [H, NBW], dt.float32, kind="ExternalOutput")
    u_own_lo = nc.dram_tensor("u_own_lo", [SPLIT, H], dt.bfloat16)
    u_own_hi = nc.dram_tensor("u_own_hi", [3200, H], dt.bfloat16)
    u_table_lo = nc.dram_tensor("u_table_lo", [NC * SPLIT, H], dt.bfloat16, addr_space="Shared")
    u_table_hi = nc.dram_tensor("u_table_hi", [NC * HI_R, H], dt.bfloat16, addr_space="Shared")

    from contextlib import ExitStack
    with tile.TileContext(nc) as tc, ExitStack() as ctx:
        pools = {}
        for nm, bufs, space in [
            ("const", 1, "SBUF"), ("idxr", 1, "SBUF"), ("idxs", 4, "SBUF"),
            ("g", 6, "SBUF"), ("vsl", 3, "SBUF"), ("y", 4, "SBUF"),
            ("t", 6, "SBUF"), ("S", 8, "SBUF"), ("S0", 3, "SBUF"), ("acc", 1, "SBUF"),
            ("h", 2, "SBUF"), ("scr", 2, "SBUF"), ("unm", 1, "SBUF"),
            ("small", 4, "SBUF"), ("oc", 3, "SBUF"),
            ("psw", 2, "PSUM"), ("ptr", 2, "PSUM"), ("p512", 2, "PSUM"), ("pt0", 2, "PSUM"),
        ]:
            pools[nm] = ctx.enter_context(tc.tile_pool(name=nm, bufs=bufs, space=space))
        env = dict(nc=nc, tc=tc, pools=pools, P=P, out_ext=out_ext,
                   u_own_lo=u_own_lo, u_own_hi=u_own_hi,
                   u_table_lo=u_table_lo, u_table_hi=u_table_hi,
                   sched=sched, dt=dt)
        _emit_program(env)

    nc.compile()
    return nc


# --------------------------------------------------------------------- entry

def kernel(gate_type, edge_src, edge_dst, edge_w, emb, W1_0, W2_0, b2_0,
           W1_rest, W2_rest, b2_rest):
    global LAST_EXEC_NS
    key = hashlib.sha1(
        np.ascontiguousarray(np.asarray(edge_dst, dtype=np.int64)).tobytes()
        + np.ascontiguousarray(np.asarray(edge_src, dtype=np.int64)).tobytes()
    ).hexdigest()

    sched, per_core = _schedule_and_arrays(
        gate_type, edge_src, edge_dst, edge_w, emb, W1_0, W2_0, b2_0,
        W1_rest, W2_rest, b2_rest)

    if key in _cache and _cache[key][1]["NT"] == sched["NT"]:
        nc = _cache[key][0]
    else:
        nc = _build_nc(sched)
        _cache.clear()
        _cache[key] = (nc, sched)

    res = run_bass_kernel_spmd(nc, per_core, core_ids=list(range(NC)),
                               trace=PROFILE)
    LAST_EXEC_NS = res.exec_time_ns

    out = np.empty((N_NODES, H), np.float32)
    for c in range(NC):
        out[c * NPC:(c + 1) * NPC] = res.results[c]["out"][:, :NPC].T
    return out
